# revision 39
# baseline (speedup 1.0000x reference)
"""Trainium2 Bass kernel for nn_MOTASG_KO_Reg (ragged graph-conv KO regression).

Strategy (8 NeuronCores, data-parallel over node rows):
  - N=16384 nodes = 16 batch samples x 1024 entities. Core c owns rows
    [2048c, 2048c+2048) = batch samples 2c, 2c+1.
  - Activations kept feature-major ("transposed", [feat, rows]) on chip so
    every linear is a native PE matmul; row-major outputs obtained by using
    the activation as lhsT instead of the weight.
  - name/desc embedding path is tiled x16 in the reference: computed once on
    128 entities/core, AllGathered (cross_nd).
  - gconv segment-sums: m (=x@W) computed per-core row-major, cast fp16,
    AllGathered; incoming edges per destination tile are fetched with
    dma_gather (128-row chunks) and scatter-added on the TensorEngine via
    host-prepared one-hot selection matrices, accumulating in PSUM.
    Self term + real edges are all just "edges".
  - z is never materialized: m2 = zpre@enc_W + u@enc_W  (zpre = x_c + pre-part,
    u = lrelu(gconv1)).
  - Final gconv2 evaluated only at the 128 KO slots/core; readout (gate +
    softmax + weighted sum + regression) done fully on-core -> [2] floats.
"""

import functools
import numpy as np

import concourse.bacc as bacc
import concourse.mybir as mybir
import concourse.tile as tile
from concourse import bass
from concourse.bass_utils import run_bass_kernel_spmd
from concourse.masks import make_identity

# problem constants (hardcoded per harness contract)
NE, B, KO = 1024, 16, 64
TX, OM, D = 768, 511, 512
N = NE * B            # 16384
NCORE = 8
R = N // NCORE        # 2048 rows per core
NT = R // 128         # 16 row tiles per core
SLOPE = 0.3
F32 = mybir.dt.float32
F16 = mybir.dt.float16
I16 = mybir.dt.int16
AX = mybir.AxisListType.X
ALU = mybir.AluOpType
ACTF = mybir.ActivationFunctionType

WAVE = 8  # gather chunks per dma_gather call
DEBUG = False  # adds intermediate-dump outputs
TRACE = False  # profile via NTFF
TRACE_KW = None


# ---------------------------------------------------------------------------
# host-side edge preparation
# ---------------------------------------------------------------------------

def _chunk_edges_per_tile(src, dstl, ntiles, nch_per_tile):
    """Sort (src->dst_local) edges into per-destination-tile 128-edge chunks.

    Returns idx [ntiles*nch, 128] int16 (gather row ids, 0-padded) and
    dstv [ntiles*nch, 128] f32 (dst slot within tile, -2 for padding)."""
    C = ntiles * nch_per_tile
    idx = np.zeros((C, 128), np.int16)
    dstv = np.full((C, 128), -2.0, np.float32)
    t_of = dstl >> 7
    for t in range(ntiles):
        m = t_of == t
        s = src[m]
        d = (dstl[m] - (t << 7)).astype(np.float32)
        n = len(s)
        assert n <= nch_per_tile * 128, (n, nch_per_tile)
        base = t * nch_per_tile
        full, rem = divmod(n, 128)
        for j in range(full):
            idx[base + j] = s[j * 128:(j + 1) * 128]
            dstv[base + j] = d[j * 128:(j + 1) * 128]
        if rem:
            idx[base + full, :rem] = s[full * 128:]
            dstv[base + full, :rem] = d[full * 128:]
    return idx, dstv


def _wrap_idx_waves(idx_chunks):
    """[C,128] int16 -> [128, C*8] wrapped per WAVE-chunk call for dma_gather."""
    C = idx_chunks.shape[0]
    assert C % WAVE == 0
    cols = []
    for w in range(C // WAVE):
        lin = idx_chunks[w * WAVE:(w + 1) * WAVE].reshape(-1)  # 1024 linear
        wrapped = lin.reshape(-1, 16).T                        # [16, 64]
        cols.append(np.tile(wrapped, (8, 1)))                  # [128, 64]
    return np.ascontiguousarray(np.concatenate(cols, axis=1))


def _sel_from_dstv(dstv):
    """[C,128] f32 (slot or -2) -> [C*128, 128] f16 one-hot selection."""
    C = dstv.shape[0]
    sel = (dstv[:, :, None] == np.arange(128, dtype=np.float32)[None, None, :])
    return np.ascontiguousarray(sel.reshape(C * 128, 128).astype(np.float16))


def _pad_w(w, rows, cols):
    out = np.zeros((rows, cols), np.float32)
    out[:w.shape[0], :w.shape[1]] = w
    return out


# ---------------------------------------------------------------------------
# program builder (structure depends only on chunk counts)
# ---------------------------------------------------------------------------

@functools.lru_cache(maxsize=4)
def _build(nch1, nch2):
    C1 = NT * nch1           # gconv1 chunks (multiple of 8 since NT=16)
    C2 = nch2                # gconv2 chunks (padded to multiple of 8 by host)
    nc = bacc.Bacc("TRN2")

    def din(name, shape, dtype=F32):
        return nc.dram_tensor(name, shape, dtype, kind="ExternalInput")

    x_t = din("x_t", [512, R])                  # [x | ko]^T
    pre_t_d = din("pre_t", [512, R])            # [pre_x | ko]^T
    ndemb = din("ndemb", [2 * TX, 128])         # name^T ; desc^T (128 entities)
    name_W = din("name_W", [TX, TX])
    desc_W = din("desc_W", [TX, TX])
    omic_W = din("omic_W", [512, 512])
    fus_nd = din("fus_nd", [2 * TX, 512])
    fus_om = din("fus_om", [512, 512])
    ienc_W = din("ienc_W", [512, 512])
    pre_W = din("pre_W", [512, 512])
    enc_W = din("enc_W", [512, 512])
    gate_W1 = din("gate_W1", [512, 512])
    gw2reg = din("gw2reg", [128, 8])  # [p, 2a+c] = W[a*128+p, c], c in {W2, reg}
    bias_pf = din("bias_pf", [128, 26])   # cols: name6 desc6 omic4 ienc4 enc4 scal2(b2,regb)
    bias_rows = din("bias_rows", [96, 512])  # partitions 0/32/64: fus_b, pre_b, gate_b1
    idx1_d = din("idx1", [128, C1 * 8], I16)
    sel1_d = din("sel1", [C1 * 128, 128], F16)
    idx2_d = din("idx2", [128, C2 * 8], I16)
    sel2_d = din("sel2", [C2 * 128, 128], F16)
    out_d = nc.dram_tensor("out", [1, 2], F32, kind="ExternalOutput")
    if DEBUG:
        dbg_cross = nc.dram_tensor("dbg_cross", [512, R], F32, kind="ExternalOutput")
        dbg_m1 = nc.dram_tensor("dbg_m1", [R, 512], F16, kind="ExternalOutput")
        dbg_m2 = nc.dram_tensor("dbg_m2", [R, 512], F16, kind="ExternalOutput")
        dbg_zk = nc.dram_tensor("dbg_zk", [128, 512], F32, kind="ExternalOutput")
        dbg_ut = nc.dram_tensor("dbg_ut", [512, R], F32, kind="ExternalOutput")
        dbg_ro = nc.dram_tensor("dbg_ro", [2, 128], F32, kind="ExternalOutput")
        dbg_fusom = nc.dram_tensor("dbg_fusom", [512, R], F32, kind="ExternalOutput")
        dbg_agnd = nc.dram_tensor("dbg_agnd", [NCORE * 512, 128], F32,
                                  kind="ExternalOutput")

    # internal DRAM for collectives
    agnd_in = nc.dram_tensor("agnd_in", [512, 128], F32)
    agnd_out = nc.dram_tensor("agnd_out", [NCORE * 512, 128], F32, addr_space="Shared")
    ag1_in = nc.dram_tensor("ag1_in", [R, 512], F16)
    ag1_out = nc.dram_tensor("ag1_out", [N, 512], F16, addr_space="Shared")
    ag2_in = nc.dram_tensor("ag2_in", [R, 512], F16)
    m2a_d = nc.dram_tensor("m2a_d", [R, 512], F16)
    ag2_out = nc.dram_tensor("ag2_out", [N, 512], F16, addr_space="Shared")
    RG = [list(range(NCORE))]

    with tile.TileContext(nc) as tc:
        with (
            tc.tile_pool(name="pbig", bufs=8) as pbig,       # [128, R] f32
            tc.tile_pool(name="pmed", bufs=1) as pmed,       # [128, 512]-ish
            tc.tile_pool(name="pw", bufs=1) as pw,
            tc.tile_pool(name="pg", bufs=1) as pg,
            tc.tile_pool(name="psc", bufs=1) as psc,
            tc.tile_pool(name="pp", bufs=1, space="PSUM") as pp,
        ):
            # ---- small constants -------------------------------------------------
            bpf = psc.tile([128, 26], F32, tag="bpf", bufs=1)
            nc.sync.dma_start(out=bpf[:], in_=bias_pf[:])
            brow_f = psc.tile([1, 512], F32, tag="brow_f", bufs=1)
            nc.sync.dma_start(out=brow_f[:], in_=bias_rows[0:1, :])
            brow_p = psc.tile([1, 512], F32, tag="brow_p", bufs=1)
            nc.sync.dma_start(out=brow_p[:], in_=bias_rows[32:33, :])
            brow_g = psc.tile([1, 512], F32, tag="brow_g", bufs=1)
            nc.sync.dma_start(out=brow_g[:], in_=bias_rows[64:65, :])
            ones = psc.tile([1, 512], F32, tag="ones", bufs=1)
            nc.vector.memset(ones[:], 1.0)
            ident = psc.tile([128, 128], F32, tag="ident", bufs=1)
            make_identity(nc, ident[:])
            idx1 = psc.tile([128, C1 * 8], I16, tag="idx1", bufs=1)
            nc.sync.dma_start(out=idx1[:], in_=idx1_d[:])
            idx2 = psc.tile([128, C2 * 8], I16, tag="idx2", bufs=1)
            nc.sync.dma_start(out=idx2[:], in_=idx2_d[:])

            # ---- load big activations -------------------------------------------
            xt = []
            for k in range(4):
                t = pbig.tile([128, R], F32, tag="bigA", bufs=8, name=f"xt{k}")
                nc.sync.dma_start(out=t[:], in_=x_t[128 * k:128 * (k + 1), :])
                xt.append(t)

            # ---- phase ND: name/desc -> cross_nd (128 entities) -----------------
            # nd_out^T[fo, e] = lrelu(W^T @ emb^T + b); then fus_nd part.
            nd_act = []   # 12 tiles [128, 128] (6 name + 6 desc)
            for half in range(2):
                W_d = name_W if half == 0 else desc_W
                embs = []
                for ki in range(6):
                    emb_k = psc.tile([128, 128], F32, tag="emb", bufs=7,
                                     name=f"emb{half}_{ki}")
                    nc.sync.dma_start(
                        out=emb_k[:],
                        in_=ndemb[half * TX + 128 * ki: half * TX + 128 * (ki + 1), :])
                    embs.append(emb_k)
                for mo in range(6):
                    ps = pp.tile([128, 512], F32, tag="ps_mm", bufs=2, space="PSUM")
                    for ki in range(6):
                        wblk = pw.tile([128, 128], F32, tag="wnd", bufs=4)
                        nc.sync.dma_start(
                            out=wblk[:],
                            in_=W_d[128 * ki:128 * (ki + 1), 128 * mo:128 * (mo + 1)])
                        nc.tensor.matmul(ps[:, :128], lhsT=wblk[:], rhs=embs[ki][:],
                                         start=(ki == 0), stop=(ki == 5))
                    a = psc.tile([128, 128], F32, tag="ndact", bufs=12,
                                 name=f"ndact{half}_{mo}")
                    btile = bpf[:, 6 * half + mo:6 * half + mo + 1]
                    tmpnd = psc.tile([128, 128], F32, tag="tmpnd", bufs=2)
                    nc.vector.tensor_scalar(out=tmpnd[:], in0=ps[:, :128],
                                            scalar1=btile, scalar2=SLOPE,
                                            op0=ALU.add, op1=ALU.mult)
                    nc.vector.tensor_scalar(out=a[:], in0=ps[:, :128],
                                            scalar1=btile, scalar2=None, op0=ALU.add)
                    nc.vector.tensor_tensor(out=a[:], in0=a[:], in1=tmpnd[:],
                                            op=ALU.max)
                    nd_act.append(a)
            # cross_nd^T [512, 128] = fus_nd^T @ ndact (+0 bias; fus_b added in fus-om)
            ndres = []
            for mo in range(4):
                ps = pp.tile([128, 512], F32, tag="ps_mm", bufs=2, space="PSUM")
                for ki in range(12):
                    wblk = pw.tile([128, 128], F32, tag="wnd", bufs=4)
                    nc.sync.dma_start(
                        out=wblk[:],
                        in_=fus_nd[128 * ki:128 * (ki + 1), 128 * mo:128 * (mo + 1)])
                    nc.tensor.matmul(ps[:, :128], lhsT=wblk[:], rhs=nd_act[ki][:],
                                     start=(ki == 0), stop=(ki == 11))
                r_ = psc.tile([128, 128], F32, tag="ndres", bufs=4)
                nc.vector.tensor_copy(out=r_[:], in_=ps[:, :128])
                ndres.append(r_)
            # bounce to DRAM + AllGather over entity blocks
            for mo in range(4):
                nc.sync.dma_start(out=agnd_in[128 * mo:128 * (mo + 1), :], in_=ndres[mo][:])
            nc.gpsimd.collective_compute(
                "AllGather", ALU.bypass, replica_groups=RG,
                ins=[agnd_in[:]], outs=[agnd_out[:]])
            if DEBUG:
                nc.sync.dma_start(out=dbg_agnd[:], in_=agnd_out[:])

            # ---- phase A: omic + fus -> cross_c^T [512, R] ----------------------
            womic = [pw.tile([128, 512], F32, tag="wres", bufs=12, name=f"womic{k}")
                     for k in range(4)]
            for k in range(4):
                nc.sync.dma_start(out=womic[k][:], in_=omic_W[128 * k:128 * (k + 1), :])
            wfom = [pw.tile([128, 512], F32, tag="wres", bufs=12, name=f"wfom{k}")
                    for k in range(4)]
            for k in range(4):
                nc.sync.dma_start(out=wfom[k][:], in_=fus_om[128 * k:128 * (k + 1), :])
            cross = [pbig.tile([128, R], F32, tag="bigA", bufs=8, name=f"cross{k}")
                     for k in range(4)]
            for j in range(4):  # 512-row blocks
                sl = slice(512 * j, 512 * (j + 1))
                om_j = []
                for k in range(4):
                    ps = pp.tile([128, 512], F32, tag="ps_mm", bufs=2, space="PSUM")
                    for ki in range(4):
                        nc.tensor.matmul(ps[:], lhsT=womic[ki][:, 128 * k:128 * (k + 1)],
                                         rhs=xt[ki][:, sl], start=(ki == 0), stop=(ki == 3))
                    a = pmed.tile([128, 512], F32, tag="omj", bufs=5)
                    btile = bpf[:, 12 + k:13 + k]
                    tmpom = pmed.tile([128, 512], F32, tag="tmpom", bufs=2)
                    nc.vector.tensor_scalar(out=tmpom[:], in0=ps[:],
                                            scalar1=btile, scalar2=SLOPE,
                                            op0=ALU.add, op1=ALU.mult)
                    nc.vector.tensor_scalar(out=a[:], in0=ps[:],
                                            scalar1=btile, scalar2=None, op0=ALU.add)
                    nc.vector.tensor_tensor(out=a[:], in0=a[:], in1=tmpom[:],
                                            op=ALU.max)
                    om_j.append(a)
                for k in range(4):
                    ps = pp.tile([128, 512], F32, tag="ps_mm", bufs=2, space="PSUM")
                    for ki in range(4):
                        nc.tensor.matmul(ps[:], lhsT=wfom[ki][:, 128 * k:128 * (k + 1)],
                                         rhs=om_j[ki][:], start=(ki == 0), stop=False)
                    # + fus_b (ones-row) : out[f, r] += fus_b[f]
                    nc.tensor.matmul(ps[:], lhsT=brow_f[:, 128 * k:128 * (k + 1)],
                                     rhs=ones[:], start=False, stop=True)
                    nc.vector.tensor_copy(out=cross[k][:, sl], in_=ps[:])
                    if DEBUG:
                        nc.sync.dma_start(out=dbg_fusom[128 * k:128 * (k + 1), sl],
                                          in_=cross[k][:, sl])
                    # + tiled cross_nd via accumulate-DMA from the AG buffer:
                    # entities for rows sl are ranks [4*(j%2), 4*(j%2)+4)
                    r0 = 4 * (j % 2)
                    src = agnd_out[:].rearrange(
                        "(r q p) c -> q p r c", r=NCORE, q=4)[k, :, r0:r0 + 4, :]
                    nc.gpsimd.dma_start(
                        out=cross[k][:, sl].rearrange("p (r c) -> p r c", r=4),
                        in_=src, accum_op=ALU.add)
            # overwrite feature row 511 with ko row (= x_t row 511)
            nc.sync.dma_start(out=cross[3][127:128, :], in_=x_t[511:512, :])
            if DEBUG:
                for k in range(4):
                    nc.sync.dma_start(out=dbg_cross[128 * k:128 * (k + 1), :],
                                      in_=cross[k][:])

            # ---- m1 = cross_c @ ienc_W (row-major), cast fp16, AG ---------------
            m1h = []
            wienc = [pw.tile([128, 512], F32, tag="wres", bufs=12, name=f"wienc{k}")
                     for k in range(4)]
            for k in range(4):
                nc.sync.dma_start(out=wienc[k][:], in_=ienc_W[128 * k:128 * (k + 1), :])
            for t in range(NT):
                tsl = slice(128 * t, 128 * (t + 1))
                ps = pp.tile([128, 512], F32, tag="ps_mm", bufs=2, space="PSUM")
                for ki in range(4):
                    nc.tensor.matmul(ps[:], lhsT=cross[ki][:, tsl], rhs=wienc[ki][:],
                                     start=(ki == 0), stop=(ki == 3))
                h = pmed.tile([128, 512], F16, tag="m1h", bufs=NT + 1, name=f"m1h{t}")
                nc.vector.tensor_copy(out=h[:], in_=ps[:])
                nc.sync.dma_start(out=ag1_in[tsl, :], in_=h[:])
                m1h.append(h)
            nc.gpsimd.collective_compute(
                "AllGather", ALU.bypass, replica_groups=RG,
                ins=[ag1_in[:]], outs=[ag1_out[:]])
            if DEBUG:
                nc.sync.dma_start(out=dbg_m1[:], in_=ag1_in[:])

            # ---- zpre^T = x_c^T + (pre_c @ pre_W + pre_b)^T  (in-place on xt) ---
            wpre = [pw.tile([128, 512], F32, tag="wres", bufs=12, name=f"wpre{k}")
                    for k in range(4)]
            for k in range(4):
                nc.sync.dma_start(out=wpre[k][:], in_=pre_W[128 * k:128 * (k + 1), :])
            for j in range(4):
                sl = slice(512 * j, 512 * (j + 1))
                pre_j = []
                for ki in range(4):
                    s = pmed.tile([128, 512], F32, tag="prestream", bufs=5)
                    nc.sync.dma_start(out=s[:], in_=pre_t_d[128 * ki:128 * (ki + 1), sl])
                    pre_j.append(s)
                for k in range(4):
                    ps = pp.tile([128, 512], F32, tag="ps_mm", bufs=2, space="PSUM")
                    for ki in range(4):
                        nc.tensor.matmul(ps[:], lhsT=wpre[ki][:, 128 * k:128 * (k + 1)],
                                         rhs=pre_j[ki][:], start=(ki == 0), stop=False)
                    nc.tensor.matmul(ps[:], lhsT=brow_p[:, 128 * k:128 * (k + 1)],
                                     rhs=ones[:], start=False, stop=True)
                    nc.vector.tensor_tensor(out=xt[k][:, sl], in0=xt[k][:, sl],
                                            in1=ps[:], op=ALU.add)

            # ---- m2_a = zpre @ enc_W (row-major, fp16 stash) --------------------
            wenc = [pw.tile([128, 512], F32, tag="wres", bufs=12, name=f"wenc{k}")
                    for k in range(4)]
            for k in range(4):
                nc.sync.dma_start(out=wenc[k][:], in_=enc_W[128 * k:128 * (k + 1), :])
            for t in range(NT):
                tsl = slice(128 * t, 128 * (t + 1))
                ps = pp.tile([128, 512], F32, tag="ps_mm", bufs=2, space="PSUM")
                for ki in range(4):
                    nc.tensor.matmul(ps[:], lhsT=xt[ki][:, tsl], rhs=wenc[ki][:],
                                     start=(ki == 0), stop=(ki == 3))
                h = pmed.tile([128, 512], F16, tag="m2a", bufs=3)
                nc.vector.tensor_copy(out=h[:], in_=ps[:])
                nc.sync.dma_start(out=m2a_d[tsl, :], in_=h[:])

            # ---- gconv1 scatter: gather waves + one-hot matmuls -----------------
            def scatter(ag_src, idx_t, sel_d, nchunks, tile_bounds, psum_tag):
                """Yields (tile_index, psum) after each dst-tile accumulates."""
                gbufs = {}
                sbufs = {}
                out_psums = []
                ps = None
                sel_r = sel_d[:].rearrange("(c e) d -> e c d", e=128)
                for i in range(nchunks):
                    w, slot = divmod(i, WAVE)
                    if slot == 0:
                        g = pg.tile([128, WAVE, 512], F16, tag="gath", bufs=2)
                        nc.gpsimd.dma_gather(
                            g[:], ag_src[:], idx_t[:, 64 * w:64 * (w + 1)],
                            WAVE * 128, WAVE * 128, 512)
                        gbufs[w] = g
                        sw = pg.tile([128, WAVE, 128], F16, tag="selw", bufs=2)
                        nc.sync.dma_start(
                            out=sw[:], in_=sel_r[:, WAVE * w:WAVE * (w + 1), :])
                        sbufs[w] = sw
                    t_id, first, last = tile_bounds[i]
                    if first:
                        ps = pp.tile([128, 512], F32, tag=psum_tag, bufs=2, space="PSUM")
                    nc.tensor.matmul(ps[:], lhsT=sbufs[w][:, slot, :],
                                     rhs=gbufs[w][:, slot, :],
                                     start=first, stop=last)
                    if last:
                        out_psums.append((t_id, ps))
                return out_psums

            bounds1 = []
            for i in range(C1):
                t_id, j = divmod(i, nch1)
                bounds1.append((t_id, j == 0, j == nch1 - 1))
            seg1 = scatter(ag1_out, idx1, sel1_d, C1, bounds1, "ps_seg")

            # u^T via PE transpose + lrelu(+ienc_b):  u_t[k][:, 128t:...] fp32
            ut = [pbig.tile([128, R], F32, tag="bigA", bufs=8, name=f"ut{k}")
                  for k in range(4)]
            for q in range(4):          # quads of 4 dst tiles
                pst = [pp.tile([128, 512], F32, tag="ps_ut", bufs=4, space="PSUM",
                               name=f"pst{q}_{k_}") for k_ in range(4)]
                for tt in range(4):
                    t_id, ps = seg1[4 * q + tt]
                    useg = pmed.tile([128, 512], F32, tag="useg", bufs=3)
                    nc.vector.tensor_tensor(out=useg[:], in0=ps[:],
                                            in1=m1h[t_id][:], op=ALU.add)
                    for k in range(4):
                        nc.tensor.transpose(
                            out=pst[k][:, 128 * tt:128 * (tt + 1)],
                            in_=useg[:, 128 * k:128 * (k + 1)], identity=ident[:])
                for k in range(4):
                    sl = slice(512 * q, 512 * (q + 1))
                    tmp = pmed.tile([128, 512], F32, tag="lrtmp", bufs=2)
                    nc.vector.tensor_scalar(
                        out=tmp[:], in0=pst[k][:], scalar1=bpf[:, 16 + k:17 + k],
                        scalar2=SLOPE, op0=ALU.add, op1=ALU.mult)
                    nc.vector.tensor_scalar(
                        out=ut[k][:, sl], in0=pst[k][:], scalar1=bpf[:, 16 + k:17 + k],
                        scalar2=None, op0=ALU.add)
                    nc.vector.tensor_tensor(
                        out=ut[k][:, sl], in0=ut[k][:, sl], in1=tmp[:], op=ALU.max)

            # ---- m2 = m2a + u @ enc_W -> fp16 -> AG2 ----------------------------
            for t in range(NT):
                tsl = slice(128 * t, 128 * (t + 1))
                ps = pp.tile([128, 512], F32, tag="ps_mm", bufs=2, space="PSUM")
                for ki in range(4):
                    nc.tensor.matmul(ps[:], lhsT=ut[ki][:, tsl], rhs=wenc[ki][:],
                                     start=(ki == 0), stop=(ki == 3))
                m2a_t = pmed.tile([128, 512], F16, tag="m2a", bufs=3)
                nc.sync.dma_start(out=m2a_t[:], in_=m2a_d[tsl, :])
                h = pmed.tile([128, 512], F16, tag="m2h", bufs=3)
                nc.vector.tensor_tensor(out=h[:], in0=ps[:], in1=m2a_t[:], op=ALU.add)
                nc.sync.dma_start(out=ag2_in[tsl, :], in_=h[:])
            nc.gpsimd.collective_compute(
                "AllGather", ALU.bypass, replica_groups=RG,
                ins=[ag2_in[:]], outs=[ag2_out[:]])
            if DEBUG:
                nc.sync.dma_start(out=dbg_m2[:], in_=ag2_in[:])
                for k in range(4):
                    nc.sync.dma_start(out=dbg_ut[128 * k:128 * (k + 1), :],
                                      in_=ut[k][:])

            # ---- gconv2 at the 128 KO slots -------------------------------------
            bounds2 = [(0, i == 0, i == C2 - 1) for i in range(C2)]
            seg2 = scatter(ag2_out, idx2, sel2_d, C2, bounds2, "ps_seg")
            _, ps_zk = seg2[0]
            zkr = pmed.tile([128, 512], F32, tag="useg", bufs=3)
            nc.vector.tensor_copy(out=zkr[:], in_=ps_zk[:])
            if DEBUG:
                nc.sync.dma_start(out=dbg_zk[:], in_=zkr[:])
            ps_zt = pp.tile([128, 512], F32, tag="ps_ut", bufs=4, space="PSUM")
            for k in range(4):
                nc.tensor.transpose(out=ps_zt[:, 128 * k:128 * (k + 1)],
                                    in_=zkr[:, 128 * k:128 * (k + 1)], identity=ident[:])
            zkt = pmed.tile([128, 512], F32, tag="zkt", bufs=1)
            for k in range(4):
                sl = slice(128 * k, 128 * (k + 1))
                tmp = pmed.tile([128, 128], F32, tag="lrtmp2", bufs=2)
                nc.vector.tensor_scalar(
                    out=tmp[:], in0=ps_zt[:, sl], scalar1=bpf[:, 20 + k:21 + k],
                    scalar2=SLOPE, op0=ALU.add, op1=ALU.mult)
                nc.vector.tensor_scalar(
                    out=zkt[:, sl], in0=ps_zt[:, sl], scalar1=bpf[:, 20 + k:21 + k],
                    scalar2=None, op0=ALU.add)
                nc.vector.tensor_tensor(
                    out=zkt[:, sl], in0=zkt[:, sl], in1=tmp[:], op=ALU.max)

            # ---- readout --------------------------------------------------------
            wg1 = [pw.tile([128, 512], F32, tag="wres", bufs=12, name=f"wg1{k}")
                   for k in range(4)]
            for k in range(4):
                nc.sync.dma_start(out=wg1[k][:], in_=gate_W1[128 * k:128 * (k + 1), :])
            w2r = psc.tile([128, 8], F32, tag="w2r", bufs=1)  # [512,2] as 4x[128,2]
            nc.sync.dma_start(out=w2r[:], in_=gw2reg[:])
            s1t = pmed.tile([128, 512], F32, tag="s1t", bufs=1)
            for ko_ in range(4):
                ps = pp.tile([128, 512], F32, tag="ps_mm", bufs=2, space="PSUM")
                for ki in range(4):
                    nc.tensor.matmul(ps[:, :128],
                                     lhsT=wg1[ki][:, 128 * ko_:128 * (ko_ + 1)],
                                     rhs=zkt[:, 128 * ki:128 * (ki + 1)],
                                     start=(ki == 0), stop=False)
                nc.tensor.matmul(ps[:, :128],
                                 lhsT=brow_g[:, 128 * ko_:128 * (ko_ + 1)],
                                 rhs=ones[:, :128], start=False, stop=True)
                nc.scalar.activation(s1t[:, 128 * ko_:128 * (ko_ + 1)], ps[:, :128],
                                     ACTF.Tanh)
            # scores^T [1,128] and t^T [1,128]
            ps_sc = pp.tile([128, 512], F32, tag="ps_mm", bufs=2, space="PSUM")
            for ki in range(4):
                nc.tensor.matmul(ps_sc[:1, :128], lhsT=w2r[:, 2 * ki:2 * ki + 1],
                                 rhs=s1t[:, 128 * ki:128 * (ki + 1)],
                                 start=(ki == 0), stop=(ki == 3))
            ps_tr = pp.tile([128, 512], F32, tag="ps_seg", bufs=2, space="PSUM")
            for ki in range(4):
                nc.tensor.matmul(ps_tr[:1, :128], lhsT=w2r[:, 2 * ki + 1:2 * ki + 2],
                                 rhs=zkt[:, 128 * ki:128 * (ki + 1)],
                                 start=(ki == 0), stop=(ki == 3))
            erow = psc.tile([1, 128], F32, tag="erow", bufs=1)
            nc.scalar.activation(erow[:], ps_sc[:1, :128], ACTF.Exp,
                                 bias=bpf[:1, 24:25])
            etrow = psc.tile([1, 128], F32, tag="etrow", bufs=1)
            nc.vector.tensor_tensor(out=etrow[:], in0=erow[:], in1=ps_tr[:1, :128],
                                    op=ALU.mult)
            if DEBUG:
                trow = psc.tile([1, 128], F32, tag="trow", bufs=1)
                nc.vector.tensor_copy(out=trow[:], in_=ps_tr[:1, :128])
                nc.sync.dma_start(out=dbg_ro[0:1, :], in_=erow[:])
                nc.sync.dma_start(out=dbg_ro[1:2, :], in_=trow[:])
            sums = psc.tile([1, 4], F32, tag="sums", bufs=1)
            nc.vector.tensor_reduce(out=sums[:, 0:2],
                                    in_=etrow[:].rearrange("p (g x) -> p g x", g=2),
                                    axis=AX, op=ALU.add)
            nc.vector.tensor_reduce(out=sums[:, 2:4],
                                    in_=erow[:].rearrange("p (g x) -> p g x", g=2),
                                    axis=AX, op=ALU.add)
            res = psc.tile([1, 4], F32, tag="res", bufs=1)
            nc.vector.reciprocal(out=res[:, 2:4], in_=sums[:, 2:4])
            nc.vector.tensor_tensor(out=res[:, 0:2], in0=sums[:, 0:2],
                                    in1=res[:, 2:4], op=ALU.mult)
            nc.vector.tensor_scalar(out=res[:, 0:2], in0=res[:, 0:2],
                                    scalar1=bpf[:1, 25:26], scalar2=None, op0=ALU.add)
            nc.sync.dma_start(out=out_d[:], in_=res[:, 0:2])

    nc.compile()
    return nc


def _ensure_ntff_hook():
    """Inject antenv.axon_hooks (absent in this image) so trace=True works."""
    import sys, types
    try:
        from antenv.axon_hooks import get_axon_ntff_profile_hook  # noqa
        return
    except ImportError:
        pass
    import antenv
    mod = types.ModuleType("antenv.axon_hooks")
    _state = {"hook": None}
    mod.set_axon_ntff_profile_hook = lambda h: _state.__setitem__("hook", h)
    mod.get_axon_ntff_profile_hook = lambda: _state["hook"]
    sys.modules["antenv.axon_hooks"] = mod
    antenv.axon_hooks = mod
    from trn_agent_boot.trn_boot import _ntff_profile_via_ctypes
    mod.set_axon_ntff_profile_hook(
        _ntff_profile_via_ctypes("/opt/axon/libaxon_pjrt.so"))


# ---------------------------------------------------------------------------
# host wrapper
# ---------------------------------------------------------------------------

def kernel(**inputs):
    x = np.asarray(inputs["x"], np.float32)
    pre_x = np.asarray(inputs["pre_x"], np.float32)
    edge_index = np.asarray(inputs["edge_index"], np.int64)
    internal_edge_index = np.asarray(inputs["internal_edge_index"], np.int64)
    name_emb = np.asarray(inputs["name_embeddings"], np.float32)
    desc_emb = np.asarray(inputs["desc_embeddings"], np.float32)
    ko_mask = np.asarray(inputs["ko_mask"], np.int64)
    bkm = np.asarray(inputs["batch_ko_masks"], np.int64)
    name_W = np.asarray(inputs["name_W"], np.float32); name_b = np.asarray(inputs["name_b"], np.float32)
    desc_W = np.asarray(inputs["desc_W"], np.float32); desc_b = np.asarray(inputs["desc_b"], np.float32)
    omic_W = np.asarray(inputs["omic_W"], np.float32); omic_b = np.asarray(inputs["omic_b"], np.float32)
    fus_W = np.asarray(inputs["fus_W"], np.float32); fus_b = np.asarray(inputs["fus_b"], np.float32)
    pre_W = np.asarray(inputs["pre_W"], np.float32); pre_b = np.asarray(inputs["pre_b"], np.float32)
    ienc_W = np.asarray(inputs["ienc_W"], np.float32); ienc_b = np.asarray(inputs["ienc_b"], np.float32)
    enc_W = np.asarray(inputs["enc_W"], np.float32); enc_b = np.asarray(inputs["enc_b"], np.float32)
    gate_W1 = np.asarray(inputs["gate_W1"], np.float32); gate_b1 = np.asarray(inputs["gate_b1"], np.float32)
    gate_W2 = np.asarray(inputs["gate_W2"], np.float32); gate_b2 = np.asarray(inputs["gate_b2"], np.float32)
    reg_W = np.asarray(inputs["reg_W"], np.float32); reg_b = np.asarray(inputs["reg_b"], np.float32)

    ko_feat = np.zeros(N, np.float32)
    ko_feat[ko_mask] = 1.0

    # ---- per-core edge structures -----------------------------------------
    s1_all, d1_all = internal_edge_index[0], internal_edge_index[1]
    per_core_1 = []
    max_nch1 = 0
    for c in range(NCORE):
        lo, hi = R * c, R * (c + 1)
        m = (d1_all >= lo) & (d1_all < hi)
        src = s1_all[m]
        dstl = d1_all[m] - lo
        per_core_1.append((src, dstl))
        cnt = np.bincount(dstl >> 7, minlength=NT)
        max_nch1 = max(max_nch1, int(np.ceil(cnt.max() / 128)))
    nch1 = max_nch1

    s2_all, d2_all = edge_index[0], edge_index[1]
    per_core_2 = []
    max_e2 = 0
    slot_rows_all = []
    for c in range(NCORE):
        slot_rows = np.concatenate(
            [(2 * c + i) * NE + bkm[2 * c + i] for i in range(2)])  # [128] global
        slot_rows_all.append(slot_rows)
        row2slots = {}
        for s_, r_ in enumerate(slot_rows):
            row2slots.setdefault(int(r_), []).append(s_)
        m = np.isin(d2_all, slot_rows)
        su, dv = s2_all[m], d2_all[m]
        ss = [np.int64(r_) for r_ in slot_rows]   # self edges src
        ds = list(range(128))
        for u, v in zip(su, dv):
            for s_ in row2slots[int(v)]:
                ss.append(u); ds.append(s_)
        src = np.array(ss, np.int64); dstl = np.array(ds, np.int64)
        per_core_2.append((src, dstl))
        max_e2 = max(max_e2, len(src))
    nch2 = -(-max_e2 // 128)
    nch2 = -(-nch2 // WAVE) * WAVE  # pad to wave multiple

    nc = _build(nch1, nch2)

    # ---- shared weight payloads -------------------------------------------
    omic_Wp = _pad_w(omic_W, 512, 512)
    fus_ndp = _pad_w(fus_W[:2 * TX], 2 * TX, 512)
    fus_omp = _pad_w(fus_W[2 * TX:], 512, 512)
    bias_pf = np.zeros((128, 26), np.float32)
    bias_pf[:, 0:6] = name_b.reshape(6, 128).T
    bias_pf[:, 6:12] = desc_b.reshape(6, 128).T
    bias_pf[:, 12:16] = _pad_w(omic_b[:, None], 512, 1).reshape(4, 128).T
    bias_pf[:, 16:20] = ienc_b.reshape(4, 128).T
    bias_pf[:, 20:24] = enc_b.reshape(4, 128).T
    bias_pf[:, 24] = float(gate_b2.reshape(-1)[0])
    bias_pf[:, 25] = float(reg_b.reshape(-1)[0])
    bias_rows = np.zeros((96, 512), np.float32)
    bias_rows[0, 0:511] = fus_b
    bias_rows[32, :] = pre_b
    bias_rows[64, :] = gate_b1
    gw2reg = np.concatenate([gate_W2, reg_W], axis=1).astype(np.float32)
    gw2reg = gw2reg.reshape(4, 128, 2).transpose(1, 0, 2).reshape(128, 8)
    gw2reg = np.ascontiguousarray(gw2reg)

    shared = dict(
        name_W=name_W, desc_W=desc_W, omic_W=omic_Wp, fus_nd=fus_ndp,
        fus_om=fus_omp, ienc_W=ienc_W, pre_W=pre_W, enc_W=enc_W,
        gate_W1=gate_W1, gw2reg=gw2reg, bias_pf=bias_pf, bias_rows=bias_rows,
    )

    in_maps = []
    for c in range(NCORE):
        lo, hi = R * c, R * (c + 1)
        x_t = np.concatenate([x[lo:hi].T, ko_feat[None, lo:hi]], 0)
        pre_t = np.concatenate([pre_x[lo:hi].T, ko_feat[None, lo:hi]], 0)
        ndemb = np.concatenate(
            [name_emb[128 * c:128 * (c + 1)].T, desc_emb[128 * c:128 * (c + 1)].T], 0)
        i1, dv1 = _chunk_edges_per_tile(*per_core_1[c], NT, nch1)
        i2, dv2 = _chunk_edges_per_tile(*per_core_2[c], 1, nch2)
        in_maps.append(dict(
            x_t=np.ascontiguousarray(x_t),
            pre_t=np.ascontiguousarray(pre_t),
            ndemb=np.ascontiguousarray(ndemb),
            idx1=_wrap_idx_waves(i1),
            sel1=_sel_from_dstv(dv1),
            idx2=_wrap_idx_waves(i2),
            sel2=_sel_from_dstv(dv2),
            **shared,
        ))

    if TRACE:
        _ensure_ntff_hook()
    res = run_bass_kernel_spmd(nc, in_maps, core_ids=list(range(NCORE)),
                               trace=TRACE, **(TRACE_KW or {}))
    kernel._last = res
    out = np.zeros(B, np.float32)
    for c in range(NCORE):
        out[2 * c:2 * c + 2] = res.results[c]["out"][0]
    return out


# revision 45
# speedup vs baseline: 1.4328x; 1.4328x over previous
"""Trainium2 Bass kernel for nn_MOTASG_KO_Reg (ragged graph-conv KO regression).

Strategy (8 NeuronCores, data-parallel over node rows):
  - N=16384 nodes = 16 batch samples x 1024 entities. Core c owns rows
    [2048c, 2048c+2048) = batch samples 2c, 2c+1.
  - Activations kept feature-major ("transposed", [feat, rows]) on chip so
    every linear is a native PE matmul (fp16 operands, fp32 PSUM); row-major
    outputs obtained by using the activation as lhsT instead of the weight.
  - name/desc embedding path is tiled x16 in the reference: computed once on
    128 entities/core, AllGathered (cross_nd), folded in via accumulate-DMA.
  - gconv1 segment-sum: m1 computed per-core row-major, cast fp16,
    AllGathered; incoming edges per destination tile are fetched with
    dma_gather (128-row chunks) and scatter-added on the TensorEngine via
    host-built one-hot selection matrices, accumulating in PSUM.
  - z never materialized: m2 = zpre@enc_W + u@enc_W (zpre = x_c + pre-part,
    u = lrelu(gconv1)); m2 stays fp32 and LOCAL.
  - gconv2 evaluated only at the 1024 global KO slots, source-side: each core
    scatter-adds its own m2 rows into all slots (local fp32 gather + PE
    scatter), then ONE ReduceScatter returns each core its 128 slots.
  - Readout (gate + softmax + weighted sum + regression) on-core -> [2].
"""

import functools
import numpy as np

import concourse.bacc as bacc
import concourse.mybir as mybir
import concourse.tile as tile
from concourse import bass
from concourse.bass_utils import run_bass_kernel_spmd
from concourse.masks import make_identity

NE, B, KO = 1024, 16, 64
TX, OM, D = 768, 511, 512
N = NE * B
NCORE = 8
R = N // NCORE        # 2048 rows per core
NT = R // 128         # 16 row tiles per core
SLOPE = 0.3
F32 = mybir.dt.float32
F16 = mybir.dt.float16
I16 = mybir.dt.int16
AX = mybir.AxisListType.X
ALU = mybir.AluOpType
ACTF = mybir.ActivationFunctionType

WAVE = 16  # gather chunks per dma_gather call
WCOLS = WAVE * 8
DEBUG = False
TRACE = False
TRACE_KW = None


# ---------------------------------------------------------------------------
# host-side edge preparation
# ---------------------------------------------------------------------------

def _chunk_edges_per_tile(src, dstl, ntiles, nch_per_tile):
    """Sort (src->dst_local) into per-destination-tile 128-edge chunks."""
    C = ntiles * nch_per_tile
    idx = np.zeros((C, 128), np.int16)
    dstv = np.full((C, 128), -2.0, np.float32)
    t_of = dstl >> 7
    for t in range(ntiles):
        m = t_of == t
        s = src[m]
        d = (dstl[m] - (t << 7)).astype(np.float32)
        n = len(s)
        assert n <= nch_per_tile * 128, (n, nch_per_tile)
        base = t * nch_per_tile
        full, rem = divmod(n, 128)
        for j in range(full):
            idx[base + j] = s[j * 128:(j + 1) * 128]
            dstv[base + j] = d[j * 128:(j + 1) * 128]
        if rem:
            idx[base + full, :rem] = s[full * 128:]
            dstv[base + full, :rem] = d[full * 128:]
    return idx, dstv


def _wrap_idx_waves(idx_chunks):
    """[C,128] int16 -> [128, nwaves*128] wrapped per dma_gather call."""
    C = idx_chunks.shape[0]
    cols = []
    for w in range((C + WAVE - 1) // WAVE):
        lin = idx_chunks[w * WAVE:(w + 1) * WAVE].reshape(-1)
        wrapped = lin.reshape(-1, 16).T
        pad = np.zeros((16, WAVE * 8 - wrapped.shape[1]), np.int16)
        cols.append(np.tile(np.concatenate([wrapped, pad], 1), (8, 1)))
    return np.ascontiguousarray(np.concatenate(cols, axis=1))


def _sel_from_dstv(dstv, dt):
    C = dstv.shape[0]
    sel = (dstv[:, :, None] == np.arange(128, dtype=np.float32)[None, None, :])
    return np.ascontiguousarray(sel.reshape(C * 128, 128).astype(dt))


def _pad_w(w, rows, cols):
    out = np.zeros((rows, cols), np.float32)
    out[:w.shape[0], :w.shape[1]] = w
    return out


# ---------------------------------------------------------------------------
# program builder
# ---------------------------------------------------------------------------

@functools.lru_cache(maxsize=4)
def _build(nch1, nch2):
    """nch1: gconv1 chunks per dst tile (16 tiles); nch2: gconv2 chunks per
    slot tile (8 tiles of the 1024 global KO slots)."""
    C1 = NT * nch1
    C2 = 8 * nch2
    W1 = (C1 + WAVE - 1) // WAVE
    W2 = (C2 + WAVE - 1) // WAVE
    nc = bacc.Bacc("TRN2")

    def din(name, shape, dtype=F16):
        return nc.dram_tensor(name, shape, dtype, kind="ExternalInput")

    x_t = din("x_t", [512, R])                  # [x | ko]^T fp16
    pre_t_d = din("pre_t", [512, R])
    ndemb = din("ndemb", [2 * TX, 128])
    name_W = din("name_W", [TX, TX])
    desc_W = din("desc_W", [TX, TX])
    omic_W = din("omic_W", [512, 512])
    fus_nd = din("fus_nd", [2 * TX, 512])
    fus_om = din("fus_om", [512, 512])
    ienc_W = din("ienc_W", [512, 512])
    pre_W = din("pre_W", [512, 512])
    enc_W = din("enc_W", [512, 512])
    gate_W1 = din("gate_W1", [512, 512], F32)
    gw2reg = din("gw2reg", [128, 8], F32)
    bias_pf = din("bias_pf", [128, 26], F32)
    bias_rows = din("bias_rows", [96, 512], F32)
    idx1_d = din("idx1", [128, W1 * WCOLS], I16)
    sel1_d = din("sel1", [C1 * 128, 128], F16)
    idx2_d = din("idx2", [128, W2 * WCOLS], I16)
    sel2_d = din("sel2", [C2 * 128, 128], F16)
    out_d = nc.dram_tensor("out", [1, 2], F32, kind="ExternalOutput")

    agnd_in = nc.dram_tensor("agnd_in", [512, 128], F16)
    agnd_out = nc.dram_tensor("agnd_out", [NCORE * 512, 128], F16, addr_space="Shared")
    ag1_in = nc.dram_tensor("ag1_in", [R, 512], F16)
    ag1_out = nc.dram_tensor("ag1_out", [N, 512], F16, addr_space="Shared")
    m2_loc = nc.dram_tensor("m2_loc", [R, 512], F16)
    m2a_d = nc.dram_tensor("m2a_d", [R, 512], F16)
    rs_in = nc.dram_tensor("rs_in", [8 * 128, 512], F32)
    rs_out = nc.dram_tensor("rs_out", [128, 512], F32)
    RG = [list(range(NCORE))]

    if DEBUG:
        dbg_cross = nc.dram_tensor("dbg_cross", [512, R], F16, kind="ExternalOutput")
        dbg_m1 = nc.dram_tensor("dbg_m1", [R, 512], F16, kind="ExternalOutput")
        dbg_m2 = nc.dram_tensor("dbg_m2", [R, 512], F16, kind="ExternalOutput")
        dbg_zk = nc.dram_tensor("dbg_zk", [128, 512], F32, kind="ExternalOutput")
        dbg_ut = nc.dram_tensor("dbg_ut", [512, R], F16, kind="ExternalOutput")

    with tile.TileContext(nc) as tc:
        with (
            tc.tile_pool(name="pbig", bufs=8) as pbig,
            tc.tile_pool(name="pmed", bufs=1) as pmed,
            tc.tile_pool(name="pw", bufs=1) as pw,
            tc.tile_pool(name="pg", bufs=1) as pg,
            tc.tile_pool(name="psc", bufs=1) as psc,
            tc.tile_pool(name="pp", bufs=1, space="PSUM") as pp,
        ):
            # ---- constants ----
            bpf = psc.tile([128, 26], F32, tag="bpf", bufs=1)
            nc.sync.dma_start(out=bpf[:], in_=bias_pf[:])
            brow_g = psc.tile([1, 512], F32, tag="brow_g", bufs=1)
            nc.sync.dma_start(out=brow_g[:], in_=bias_rows[64:65, :])
            ones = psc.tile([1, 512], F32, tag="ones", bufs=1)
            nc.vector.memset(ones[:], 1.0)
            ident = psc.tile([128, 128], F32, tag="ident", bufs=1)
            make_identity(nc, ident[:])
            idx1 = psc.tile([128, W1 * WCOLS], I16, tag="idx1", bufs=1)
            nc.sync.dma_start(out=idx1[:], in_=idx1_d[:])
            idx2 = psc.tile([128, W2 * WCOLS], I16, tag="idx2", bufs=1)
            nc.sync.dma_start(out=idx2[:], in_=idx2_d[:])

            # ---- big activations (fp16) ----
            xt = []
            for k in range(4):
                t = pbig.tile([128, R], F16, tag="bigA", bufs=8, name=f"xt{k}")
                nc.sync.dma_start(out=t[:], in_=x_t[128 * k:128 * (k + 1), :])
                xt.append(t)

            # ---- ND path (128 entities) ----
            nd_act = []
            for half in range(2):
                W_d = name_W if half == 0 else desc_W
                embs = []
                for ki in range(6):
                    e_ = psc.tile([128, 128], F16, tag="emb", bufs=7,
                                  name=f"emb{half}_{ki}")
                    nc.sync.dma_start(
                        out=e_[:],
                        in_=ndemb[half * TX + 128 * ki: half * TX + 128 * (ki + 1), :])
                    embs.append(e_)
                for mo in range(6):
                    ps = pp.tile([128, 512], F32, tag="ps_mm", bufs=2, space="PSUM")
                    for ki in range(6):
                        wblk = pw.tile([128, 128], F16, tag="wnd", bufs=4)
                        nc.sync.dma_start(
                            out=wblk[:],
                            in_=W_d[128 * ki:128 * (ki + 1), 128 * mo:128 * (mo + 1)])
                        nc.tensor.matmul(ps[:, :128], lhsT=wblk[:], rhs=embs[ki][:],
                                         start=(ki == 0), stop=(ki == 5))
                    a = psc.tile([128, 128], F16, tag="ndact", bufs=12,
                                 name=f"ndact{half}_{mo}")
                    bt = bpf[:, 6 * half + mo:6 * half + mo + 1]
                    tnd = psc.tile([128, 128], F32, tag="tmpnd", bufs=2)
                    nc.vector.tensor_scalar(out=tnd[:], in0=ps[:, :128], scalar1=bt,
                                            scalar2=SLOPE, op0=ALU.add, op1=ALU.mult)
                    nc.vector.tensor_scalar(out=a[:], in0=ps[:, :128], scalar1=bt,
                                            scalar2=None, op0=ALU.add)
                    nc.vector.tensor_tensor(out=a[:], in0=a[:], in1=tnd[:], op=ALU.max)
                    nd_act.append(a)
            for mo in range(4):
                ps = pp.tile([128, 512], F32, tag="ps_mm", bufs=2, space="PSUM")
                for ki in range(12):
                    wblk = pw.tile([128, 128], F16, tag="wnd", bufs=4)
                    nc.sync.dma_start(
                        out=wblk[:],
                        in_=fus_nd[128 * ki:128 * (ki + 1), 128 * mo:128 * (mo + 1)])
                    nc.tensor.matmul(ps[:, :128], lhsT=wblk[:], rhs=nd_act[ki][:],
                                     start=(ki == 0), stop=(ki == 11))
                r_ = psc.tile([128, 128], F16, tag="ndres", bufs=4, name=f"ndres{mo}")
                nc.vector.tensor_copy(out=r_[:], in_=ps[:, :128])
                nc.sync.dma_start(out=agnd_in[128 * mo:128 * (mo + 1), :], in_=r_[:])
            nc.gpsimd.collective_compute(
                "AllGather", ALU.bypass, replica_groups=RG,
                ins=[agnd_in[:]], outs=[agnd_out[:]])

            # ---- omic + fus -> cross_c^T (fp16) ----
            womic = [pw.tile([128, 512], F16, tag="wres", bufs=12, name=f"womic{k}")
                     for k in range(4)]
            wfom = [pw.tile([128, 512], F16, tag="wres", bufs=12, name=f"wfom{k}")
                    for k in range(4)]
            for k in range(4):
                nc.sync.dma_start(out=womic[k][:], in_=omic_W[128 * k:128 * (k + 1), :])
                nc.sync.dma_start(out=wfom[k][:], in_=fus_om[128 * k:128 * (k + 1), :])
            cross = [pbig.tile([128, R], F16, tag="bigA", bufs=8, name=f"cross{k}")
                     for k in range(4)]
            for j in range(4):
                sl = slice(512 * j, 512 * (j + 1))
                om_j = []
                for k in range(4):
                    ps = pp.tile([128, 512], F32, tag="ps_mm", bufs=2, space="PSUM")
                    for ki in range(4):
                        nc.tensor.matmul(ps[:], lhsT=womic[ki][:, 128 * k:128 * (k + 1)],
                                         rhs=xt[ki][:, sl], start=(ki == 0), stop=(ki == 3))
                    a = pmed.tile([128, 512], F16, tag="omj", bufs=4)
                    bt = bpf[:, 12 + k:13 + k]
                    tom = pmed.tile([128, 512], F32, tag="tmpom", bufs=2)
                    nc.vector.tensor_scalar(out=tom[:], in0=ps[:], scalar1=bt,
                                            scalar2=SLOPE, op0=ALU.add, op1=ALU.mult)
                    nc.vector.tensor_scalar(out=a[:], in0=ps[:], scalar1=bt,
                                            scalar2=None, op0=ALU.add)
                    nc.vector.tensor_tensor(out=a[:], in0=a[:], in1=tom[:], op=ALU.max)
                    om_j.append(a)
                for k in range(4):
                    ps = pp.tile([128, 512], F32, tag="ps_mm", bufs=2, space="PSUM")
                    for ki in range(4):
                        nc.tensor.matmul(ps[:], lhsT=wfom[ki][:, 128 * k:128 * (k + 1)],
                                         rhs=om_j[ki][:], start=(ki == 0),
                                         stop=(ki == 3))
                    nc.vector.tensor_copy(out=cross[k][:, sl], in_=ps[:])
                    # + tiled cross_nd via accumulate-DMA (fus_b asserted zero)
                    r0 = 4 * (j % 2)
                    src = agnd_out[:].rearrange(
                        "(r q p) c -> q p r c", r=NCORE, q=4)[k, :, r0:r0 + 4, :]
                    nc.gpsimd.dma_start(
                        out=cross[k][:, sl].rearrange("p (r c) -> p r c", r=4),
                        in_=src, accum_op=ALU.add)
            nc.sync.dma_start(out=cross[3][127:128, :], in_=x_t[511:512, :])
            if DEBUG:
                for k in range(4):
                    nc.sync.dma_start(out=dbg_cross[128 * k:128 * (k + 1), :],
                                      in_=cross[k][:])

            # ---- m1 (row-major fp16) + AG1 ----
            wienc = [pw.tile([128, 512], F16, tag="wres", bufs=12, name=f"wienc{k}")
                     for k in range(4)]
            for k in range(4):
                nc.sync.dma_start(out=wienc[k][:], in_=ienc_W[128 * k:128 * (k + 1), :])
            for t in range(NT):
                tsl = slice(128 * t, 128 * (t + 1))
                ps = pp.tile([128, 512], F32, tag="ps_mm", bufs=2, space="PSUM")
                for ki in range(4):
                    nc.tensor.matmul(ps[:], lhsT=cross[ki][:, tsl], rhs=wienc[ki][:],
                                     start=(ki == 0), stop=(ki == 3))
                h = pmed.tile([128, 512], F16, tag="m1h", bufs=3)
                nc.vector.tensor_copy(out=h[:], in_=ps[:])
                nc.sync.dma_start(out=ag1_in[tsl, :], in_=h[:])
            nc.gpsimd.collective_compute(
                "AllGather", ALU.bypass, replica_groups=RG,
                ins=[ag1_in[:]], outs=[ag1_out[:]])
            if DEBUG:
                nc.sync.dma_start(out=dbg_m1[:], in_=ag1_in[:])

            # ---- zpre (in place on xt; pre_b asserted zero) ----
            wpre = [pw.tile([128, 512], F16, tag="wres", bufs=12, name=f"wpre{k}")
                    for k in range(4)]
            for k in range(4):
                nc.sync.dma_start(out=wpre[k][:], in_=pre_W[128 * k:128 * (k + 1), :])
            for j in range(4):
                sl = slice(512 * j, 512 * (j + 1))
                pre_j = []
                for ki in range(4):
                    s = pmed.tile([128, 512], F16, tag="prestream", bufs=4)
                    nc.sync.dma_start(out=s[:], in_=pre_t_d[128 * ki:128 * (ki + 1), sl])
                    pre_j.append(s)
                for k in range(4):
                    ps = pp.tile([128, 512], F32, tag="ps_mm", bufs=2, space="PSUM")
                    for ki in range(4):
                        nc.tensor.matmul(ps[:], lhsT=wpre[ki][:, 128 * k:128 * (k + 1)],
                                         rhs=pre_j[ki][:], start=(ki == 0),
                                         stop=(ki == 3))
                    nc.vector.tensor_tensor(out=xt[k][:, sl], in0=xt[k][:, sl],
                                            in1=ps[:], op=ALU.add)

            # ---- m2a = zpre @ enc_W (fp16 stash to DRAM) ----
            wenc = [pw.tile([128, 512], F16, tag="wres", bufs=12, name=f"wenc{k}")
                    for k in range(4)]
            for k in range(4):
                nc.sync.dma_start(out=wenc[k][:], in_=enc_W[128 * k:128 * (k + 1), :])
            for t in range(NT):
                tsl = slice(128 * t, 128 * (t + 1))
                ps = pp.tile([128, 512], F32, tag="ps_mm", bufs=2, space="PSUM")
                for ki in range(4):
                    nc.tensor.matmul(ps[:], lhsT=xt[ki][:, tsl], rhs=wenc[ki][:],
                                     start=(ki == 0), stop=(ki == 3))
                h = pmed.tile([128, 512], F16, tag="m2a", bufs=3)
                nc.vector.tensor_copy(out=h[:], in_=ps[:])
                nc.sync.dma_start(out=m2a_d[tsl, :], in_=h[:])

            # ---- generic gather+scatter ----
            def scatter(src_dram, idx_t, sel_d, sel_dt, nchunks, tile_bounds,
                        psum_tag, gbufs_n):
                gbufs = {}
                sbufs = {}
                out_psums = []
                ps = None
                sel_r = sel_d[:].rearrange("(c e) d -> e c d", e=128)
                for i in range(nchunks):
                    w, slot = divmod(i, WAVE)
                    if slot == 0:
                        nch_w = min(WAVE, nchunks - w * WAVE)
                        assert nch_w == WAVE, "waves must divide chunk count"
                        g = pg.tile([128, WAVE, 512], sel_dt, tag="gath",
                                    bufs=gbufs_n)
                        nc.gpsimd.dma_gather(
                            g[:, :nch_w, :], src_dram[:],
                            idx_t[:, WCOLS * w:WCOLS * w + nch_w * 8],
                            nch_w * 128, nch_w * 128, 512,
                            single_packet=False)
                        gbufs[w] = g
                        sw = pg.tile([128, WAVE, 128], sel_dt, tag="selw",
                                     bufs=gbufs_n)
                        nc.sync.dma_start(
                            out=sw[:, :nch_w, :],
                            in_=sel_r[:, WAVE * w:WAVE * w + nch_w, :])
                        sbufs[w] = sw
                    t_id, first, last = tile_bounds[i]
                    if first:
                        ps = pp.tile([128, 512], F32, tag=psum_tag, bufs=2,
                                     space="PSUM")
                    nc.tensor.matmul(ps[:], lhsT=sbufs[w][:, slot, :],
                                     rhs=gbufs[w][:, slot, :],
                                     start=first, stop=last)
                    if last:
                        out_psums.append((t_id, ps))
                return out_psums

            bounds1 = []
            for i in range(C1):
                t_id, j = divmod(i, nch1)
                bounds1.append((t_id, j == 0, j == nch1 - 1))
            seg1 = scatter(ag1_out, idx1, sel1_d, F16, C1, bounds1, "ps_seg", 3)

            # ---- u^T (fp16) via PE transpose + lrelu(+ienc_b) ----
            ut = [pbig.tile([128, R], F16, tag="bigA", bufs=8, name=f"ut{k}")
                  for k in range(4)]
            for q in range(4):
                pst = [pp.tile([128, 512], F32, tag="ps_ut", bufs=4, space="PSUM",
                               name=f"pst{q}_{k_}") for k_ in range(4)]
                for tt in range(4):
                    t_id, ps = seg1[4 * q + tt]
                    m1t = pmed.tile([128, 512], F16, tag="m1h", bufs=3)
                    nc.sync.dma_start(out=m1t[:],
                                      in_=ag1_in[128 * t_id:128 * (t_id + 1), :])
                    useg = pmed.tile([128, 512], F32, tag="useg", bufs=3)
                    nc.vector.tensor_tensor(out=useg[:], in0=ps[:],
                                            in1=m1t[:], op=ALU.add)
                    for k in range(4):
                        nc.tensor.transpose(
                            out=pst[k][:, 128 * tt:128 * (tt + 1)],
                            in_=useg[:, 128 * k:128 * (k + 1)], identity=ident[:])
                for k in range(4):
                    sl = slice(512 * q, 512 * (q + 1))
                    tmp = pmed.tile([128, 512], F32, tag="lrtmp", bufs=2)
                    nc.vector.tensor_scalar(
                        out=tmp[:], in0=pst[k][:], scalar1=bpf[:, 16 + k:17 + k],
                        scalar2=SLOPE, op0=ALU.add, op1=ALU.mult)
                    nc.vector.tensor_scalar(
                        out=ut[k][:, sl], in0=pst[k][:], scalar1=bpf[:, 16 + k:17 + k],
                        scalar2=None, op0=ALU.add)
                    nc.vector.tensor_tensor(
                        out=ut[k][:, sl], in0=ut[k][:, sl], in1=tmp[:], op=ALU.max)
            if DEBUG:
                for k in range(4):
                    nc.sync.dma_start(out=dbg_ut[128 * k:128 * (k + 1), :], in_=ut[k][:])

            # ---- m2 = m2a + u @ enc_W (fp32, stays local) ----
            for t in range(NT):
                tsl = slice(128 * t, 128 * (t + 1))
                ps = pp.tile([128, 512], F32, tag="ps_mm", bufs=2, space="PSUM")
                for ki in range(4):
                    nc.tensor.matmul(ps[:], lhsT=ut[ki][:, tsl], rhs=wenc[ki][:],
                                     start=(ki == 0), stop=(ki == 3))
                m2a_t = pmed.tile([128, 512], F16, tag="m2a", bufs=3)
                nc.sync.dma_start(out=m2a_t[:], in_=m2a_d[tsl, :])
                h = pmed.tile([128, 512], F16, tag="m2h", bufs=3)
                nc.vector.tensor_tensor(out=h[:], in0=ps[:], in1=m2a_t[:], op=ALU.add)
                nc.sync.dma_start(out=m2_loc[tsl, :], in_=h[:])
            if DEBUG:
                nc.sync.dma_start(out=dbg_m2[:], in_=m2_loc[:])

            # ---- gconv2: source-side partials over 1024 slots + ReduceScatter ----
            bounds2 = []
            for i in range(C2):
                t_id, j = divmod(i, nch2)
                bounds2.append((t_id, j == 0, j == nch2 - 1))
            seg2 = scatter(m2_loc, idx2, sel2_d, F16, C2, bounds2, "ps_seg", 3)
            for t_id, ps in seg2:
                pc = pmed.tile([128, 512], F32, tag="useg", bufs=3)
                nc.vector.tensor_copy(out=pc[:], in_=ps[:])
                nc.sync.dma_start(out=rs_in[128 * t_id:128 * (t_id + 1), :], in_=pc[:])
            nc.gpsimd.collective_compute(
                "ReduceScatter", ALU.add, replica_groups=RG,
                ins=[rs_in[:]], outs=[rs_out[:]])

            # ---- zk^T + readout ----
            zkr = pmed.tile([128, 512], F32, tag="useg", bufs=3)
            nc.sync.dma_start(out=zkr[:], in_=rs_out[:])
            if DEBUG:
                nc.sync.dma_start(out=dbg_zk[:], in_=zkr[:])
            ps_zt = pp.tile([128, 512], F32, tag="ps_ut", bufs=4, space="PSUM")
            for k in range(4):
                nc.tensor.transpose(out=ps_zt[:, 128 * k:128 * (k + 1)],
                                    in_=zkr[:, 128 * k:128 * (k + 1)], identity=ident[:])
            zkt = pmed.tile([128, 512], F32, tag="zkt", bufs=1)
            for k in range(4):
                sl = slice(128 * k, 128 * (k + 1))
                tmp = pmed.tile([128, 128], F32, tag="lrtmp2", bufs=2)
                nc.vector.tensor_scalar(
                    out=tmp[:], in0=ps_zt[:, sl], scalar1=bpf[:, 20 + k:21 + k],
                    scalar2=SLOPE, op0=ALU.add, op1=ALU.mult)
                nc.vector.tensor_scalar(
                    out=zkt[:, sl], in0=ps_zt[:, sl], scalar1=bpf[:, 20 + k:21 + k],
                    scalar2=None, op0=ALU.add)
                nc.vector.tensor_tensor(
                    out=zkt[:, sl], in0=zkt[:, sl], in1=tmp[:], op=ALU.max)

            wg1 = [pw.tile([128, 512], F32, tag="wres32", bufs=4, name=f"wg1{k}")
                   for k in range(4)]
            for k in range(4):
                nc.sync.dma_start(out=wg1[k][:], in_=gate_W1[128 * k:128 * (k + 1), :])
            w2r = psc.tile([128, 8], F32, tag="w2r", bufs=1)
            nc.sync.dma_start(out=w2r[:], in_=gw2reg[:])
            s1t = pmed.tile([128, 512], F32, tag="s1t", bufs=1)
            for ko_ in range(4):
                ps = pp.tile([128, 512], F32, tag="ps_mm", bufs=2, space="PSUM")
                for ki in range(4):
                    nc.tensor.matmul(ps[:, :128],
                                     lhsT=wg1[ki][:, 128 * ko_:128 * (ko_ + 1)],
                                     rhs=zkt[:, 128 * ki:128 * (ki + 1)],
                                     start=(ki == 0), stop=False)
                nc.tensor.matmul(ps[:, :128],
                                 lhsT=brow_g[:, 128 * ko_:128 * (ko_ + 1)],
                                 rhs=ones[:, :128], start=False, stop=True)
                nc.scalar.activation(s1t[:, 128 * ko_:128 * (ko_ + 1)], ps[:, :128],
                                     ACTF.Tanh)
            ps_sc = pp.tile([128, 512], F32, tag="ps_mm", bufs=2, space="PSUM")
            for ki in range(4):
                nc.tensor.matmul(ps_sc[:1, :128], lhsT=w2r[:, 2 * ki:2 * ki + 1],
                                 rhs=s1t[:, 128 * ki:128 * (ki + 1)],
                                 start=(ki == 0), stop=(ki == 3))
            ps_tr = pp.tile([128, 512], F32, tag="ps_seg", bufs=2, space="PSUM")
            for ki in range(4):
                nc.tensor.matmul(ps_tr[:1, :128], lhsT=w2r[:, 2 * ki + 1:2 * ki + 2],
                                 rhs=zkt[:, 128 * ki:128 * (ki + 1)],
                                 start=(ki == 0), stop=(ki == 3))
            erow = psc.tile([1, 128], F32, tag="erow", bufs=1)
            nc.scalar.activation(erow[:], ps_sc[:1, :128], ACTF.Exp,
                                 bias=bpf[:1, 24:25])
            etrow = psc.tile([1, 128], F32, tag="etrow", bufs=1)
            nc.vector.tensor_tensor(out=etrow[:], in0=erow[:], in1=ps_tr[:1, :128],
                                    op=ALU.mult)
            sums = psc.tile([1, 4], F32, tag="sums", bufs=1)
            nc.vector.tensor_reduce(out=sums[:, 0:2],
                                    in_=etrow[:].rearrange("p (g x) -> p g x", g=2),
                                    axis=AX, op=ALU.add)
            nc.vector.tensor_reduce(out=sums[:, 2:4],
                                    in_=erow[:].rearrange("p (g x) -> p g x", g=2),
                                    axis=AX, op=ALU.add)
            res = psc.tile([1, 4], F32, tag="res", bufs=1)
            nc.vector.reciprocal(out=res[:, 2:4], in_=sums[:, 2:4])
            nc.vector.tensor_tensor(out=res[:, 0:2], in0=sums[:, 0:2],
                                    in1=res[:, 2:4], op=ALU.mult)
            nc.vector.tensor_scalar(out=res[:, 0:2], in0=res[:, 0:2],
                                    scalar1=bpf[:1, 25:26], scalar2=None, op0=ALU.add)
            nc.sync.dma_start(out=out_d[:], in_=res[:, 0:2])

    nc.compile()
    return nc


def _ensure_ntff_hook():
    """Inject antenv.axon_hooks (absent in this image) so trace=True works."""
    import sys, types
    try:
        from antenv.axon_hooks import get_axon_ntff_profile_hook  # noqa
        return
    except ImportError:
        pass
    import antenv
    mod = types.ModuleType("antenv.axon_hooks")
    _state = {"hook": None}
    mod.set_axon_ntff_profile_hook = lambda h: _state.__setitem__("hook", h)
    mod.get_axon_ntff_profile_hook = lambda: _state["hook"]
    sys.modules["antenv.axon_hooks"] = mod
    antenv.axon_hooks = mod
    from trn_agent_boot.trn_boot import _ntff_profile_via_ctypes
    mod.set_axon_ntff_profile_hook(
        _ntff_profile_via_ctypes("/opt/axon/libaxon_pjrt.so"))


# ---------------------------------------------------------------------------
# host wrapper
# ---------------------------------------------------------------------------

def kernel(**inputs):
    f32 = lambda k: np.asarray(inputs[k], np.float32)
    x = f32("x"); pre_x = f32("pre_x")
    edge_index = np.asarray(inputs["edge_index"], np.int64)
    internal_edge_index = np.asarray(inputs["internal_edge_index"], np.int64)
    name_emb = f32("name_embeddings"); desc_emb = f32("desc_embeddings")
    ko_mask = np.asarray(inputs["ko_mask"], np.int64)
    bkm = np.asarray(inputs["batch_ko_masks"], np.int64)
    name_W = f32("name_W"); name_b = f32("name_b")
    desc_W = f32("desc_W"); desc_b = f32("desc_b")
    omic_W = f32("omic_W"); omic_b = f32("omic_b")
    fus_W = f32("fus_W"); fus_b = f32("fus_b")
    pre_W = f32("pre_W"); pre_b = f32("pre_b")
    ienc_W = f32("ienc_W"); ienc_b = f32("ienc_b")
    enc_W = f32("enc_W"); enc_b = f32("enc_b")
    gate_W1 = f32("gate_W1"); gate_b1 = f32("gate_b1")
    gate_W2 = f32("gate_W2"); gate_b2 = f32("gate_b2")
    reg_W = f32("reg_W"); reg_b = f32("reg_b")

    assert not fus_b.any() and not pre_b.any(), \
        "nonzero fus_b/pre_b not supported by this build"

    ko_feat = np.zeros(N, np.float32)
    ko_feat[ko_mask] = 1.0

    # ---- gconv1 edges (dst-sharded; self term added from local m1h) ----
    s1_all, d1_all = internal_edge_index[0], internal_edge_index[1]
    per_core_1 = []
    max_nch1 = 1
    for c in range(NCORE):
        lo, hi = R * c, R * (c + 1)
        m = (d1_all >= lo) & (d1_all < hi)
        per_core_1.append((s1_all[m], d1_all[m] - lo))
        cnt = np.bincount((d1_all[m] - lo) >> 7, minlength=NT)
        max_nch1 = max(max_nch1, int(np.ceil(cnt.max() / 128)))
    nch1 = max_nch1  # C1 = 16*nch1, always divisible by WAVE=16

    # ---- gconv2: source-sharded edges into the 1024 global KO slots ----
    slot_row = (bkm + np.arange(B)[:, None] * NE).reshape(-1)   # [1024]
    row2slots = {}
    for s_, r_ in enumerate(slot_row):
        row2slots.setdefault(int(r_), []).append(s_)
    s2_all, d2_all = edge_index[0], edge_index[1]
    m2mask = np.isin(d2_all, slot_row)
    per_core_2 = []
    max_nch2 = 1
    for c in range(NCORE):
        lo, hi = R * c, R * (c + 1)
        ss, ds = [], []
        for r_, sl_ in row2slots.items():
            if lo <= r_ < hi:
                for s_ in sl_:
                    ss.append(r_ - lo); ds.append(s_)
        mm = m2mask & (s2_all >= lo) & (s2_all < hi)
        for u, v in zip(s2_all[mm], d2_all[mm]):
            for s_ in row2slots[int(v)]:
                ss.append(int(u) - lo); ds.append(s_)
        src = np.array(ss, np.int64); dstl = np.array(ds, np.int64)
        per_core_2.append((src, dstl))
        cnt = np.bincount(dstl >> 7, minlength=8)
        max_nch2 = max(max_nch2, int(np.ceil(cnt.max() / 128)))
    nch2 = max_nch2 + (max_nch2 % 2)  # C2 = 8*nch2 divisible by WAVE=16

    nc = _build(nch1, nch2)

    f16 = np.float16
    omic_Wp = _pad_w(omic_W, 512, 512)
    fus_ndp = _pad_w(fus_W[:2 * TX], 2 * TX, 512)
    fus_omp = _pad_w(fus_W[2 * TX:], 512, 512)
    bias_pf = np.zeros((128, 26), np.float32)
    bias_pf[:, 0:6] = name_b.reshape(6, 128).T
    bias_pf[:, 6:12] = desc_b.reshape(6, 128).T
    bias_pf[:, 12:16] = _pad_w(omic_b[:, None], 512, 1).reshape(4, 128).T
    bias_pf[:, 16:20] = ienc_b.reshape(4, 128).T
    bias_pf[:, 20:24] = enc_b.reshape(4, 128).T
    bias_pf[:, 24] = float(gate_b2.reshape(-1)[0])
    bias_pf[:, 25] = float(reg_b.reshape(-1)[0])
    bias_rows = np.zeros((96, 512), np.float32)
    bias_rows[64, :] = gate_b1
    gw2 = np.concatenate([gate_W2, reg_W], axis=1).astype(np.float32)
    gw2 = np.ascontiguousarray(
        gw2.reshape(4, 128, 2).transpose(1, 0, 2).reshape(128, 8))

    shared = dict(
        name_W=name_W.astype(f16), desc_W=desc_W.astype(f16),
        omic_W=omic_Wp.astype(f16), fus_nd=fus_ndp.astype(f16),
        fus_om=fus_omp.astype(f16), ienc_W=ienc_W.astype(f16),
        pre_W=pre_W.astype(f16), enc_W=enc_W.astype(f16),
        gate_W1=gate_W1, gw2reg=gw2, bias_pf=bias_pf, bias_rows=bias_rows,
    )

    in_maps = []
    for c in range(NCORE):
        lo, hi = R * c, R * (c + 1)
        x_t = np.concatenate([x[lo:hi].T, ko_feat[None, lo:hi]], 0)
        pre_t = np.concatenate([pre_x[lo:hi].T, ko_feat[None, lo:hi]], 0)
        ndemb = np.concatenate(
            [name_emb[128 * c:128 * (c + 1)].T, desc_emb[128 * c:128 * (c + 1)].T], 0)
        i1, dv1 = _chunk_edges_per_tile(*per_core_1[c], NT, nch1)
        i2, dv2 = _chunk_edges_per_tile(*per_core_2[c], 8, nch2)
        in_maps.append(dict(
            x_t=np.ascontiguousarray(x_t).astype(f16),
            pre_t=np.ascontiguousarray(pre_t).astype(f16),
            ndemb=np.ascontiguousarray(ndemb).astype(f16),
            idx1=_wrap_idx_waves(i1),
            sel1=_sel_from_dstv(dv1, f16),
            idx2=_wrap_idx_waves(i2),
            sel2=_sel_from_dstv(dv2, f16),
            **shared,
        ))

    if TRACE:
        _ensure_ntff_hook()
    res = run_bass_kernel_spmd(nc, in_maps, core_ids=list(range(NCORE)),
                               trace=TRACE, **(TRACE_KW or {}))
    kernel._last = res
    out = np.zeros(B, np.float32)
    for c in range(NCORE):
        out[2 * c:2 * c + 2] = res.results[c]["out"][0]
    return out


# revision 46
# speedup vs baseline: 1.6528x; 1.1535x over previous
"""Trainium2 Bass kernel for nn_MOTASG_KO_Reg (ragged graph-conv KO regression).

Strategy (8 NeuronCores, data-parallel over node rows):
  - N=16384 nodes = 16 batch samples x 1024 entities. Core c owns rows
    [2048c, 2048c+2048) = batch samples 2c, 2c+1.
  - Activations kept feature-major ("transposed", [feat, rows]) on chip so
    every linear is a native PE matmul (fp16 operands, fp32 PSUM); row-major
    outputs obtained by using the activation as lhsT instead of the weight.
  - name/desc embedding path is tiled x16 in the reference: computed once on
    128 entities/core, AllGathered (cross_nd), folded in via accumulate-DMA.
  - gconv1 segment-sum: m1 computed per-core row-major, cast fp16,
    AllGathered; incoming edges per destination tile are fetched with
    dma_gather (128-row chunks) and scatter-added on the TensorEngine via
    host-built one-hot selection matrices, accumulating in PSUM.
  - z never materialized: m2 = zpre@enc_W + u@enc_W (zpre = x_c + pre-part,
    u = lrelu(gconv1)); m2 stays fp32 and LOCAL.
  - gconv2 evaluated only at the 1024 global KO slots, source-side: each core
    scatter-adds its own m2 rows into all slots (local fp32 gather + PE
    scatter), then ONE ReduceScatter returns each core its 128 slots.
  - Readout (gate + softmax + weighted sum + regression) on-core -> [2].
"""

import functools
import numpy as np

import concourse.bacc as bacc
import concourse.mybir as mybir
import concourse.tile as tile
from concourse import bass
from concourse.bass_utils import run_bass_kernel_spmd
from concourse.masks import make_identity

NE, B, KO = 1024, 16, 64
TX, OM, D = 768, 511, 512
N = NE * B
NCORE = 8
R = N // NCORE        # 2048 rows per core
NT = R // 128         # 16 row tiles per core
SLOPE = 0.3
F32 = mybir.dt.float32
F16 = mybir.dt.float16
I16 = mybir.dt.int16
AX = mybir.AxisListType.X
ALU = mybir.AluOpType
ACTF = mybir.ActivationFunctionType

WAVE = 8  # gather chunks per dma_gather call
WCOLS = WAVE * 8
DEBUG = False
TRACE = False
TRACE_KW = None


# ---------------------------------------------------------------------------
# host-side edge preparation
# ---------------------------------------------------------------------------

def _chunk_edges_per_tile(src, dstl, nch_t):
    """Sort (src->dst_local) into per-destination-tile 128-edge chunks."""
    C = sum(nch_t)
    idx = np.zeros((C, 128), np.int16)
    dstv = np.full((C, 128), -2.0, np.float32)
    t_of = dstl >> 7
    base = 0
    for t, nch in enumerate(nch_t):
        m = t_of == t
        s = src[m]
        d = (dstl[m] - (t << 7)).astype(np.float32)
        n = len(s)
        assert n <= nch * 128, (n, nch)
        full, rem = divmod(n, 128)
        for j in range(full):
            idx[base + j] = s[j * 128:(j + 1) * 128]
            dstv[base + j] = d[j * 128:(j + 1) * 128]
        if rem:
            idx[base + full, :rem] = s[full * 128:]
            dstv[base + full, :rem] = d[full * 128:]
        base += nch
    return idx, dstv


def _wrap_idx_waves(idx_chunks):
    """[C,128] int16 -> [128, nwaves*128] wrapped per dma_gather call."""
    C = idx_chunks.shape[0]
    cols = []
    for w in range((C + WAVE - 1) // WAVE):
        lin = idx_chunks[w * WAVE:(w + 1) * WAVE].reshape(-1)
        wrapped = lin.reshape(-1, 16).T
        pad = np.zeros((16, WAVE * 8 - wrapped.shape[1]), np.int16)
        cols.append(np.tile(np.concatenate([wrapped, pad], 1), (8, 1)))
    return np.ascontiguousarray(np.concatenate(cols, axis=1))


def _sel_from_dstv(dstv, dt):
    C = dstv.shape[0]
    sel = (dstv[:, :, None] == np.arange(128, dtype=np.float32)[None, None, :])
    return np.ascontiguousarray(sel.reshape(C * 128, 128).astype(dt))


def _pad_w(w, rows, cols):
    out = np.zeros((rows, cols), np.float32)
    out[:w.shape[0], :w.shape[1]] = w
    return out


# ---------------------------------------------------------------------------
# program builder
# ---------------------------------------------------------------------------

@functools.lru_cache(maxsize=4)
def _build(nch1_t, nch2_t):
    """nch1_t: gconv1 chunks per dst tile (len 16); nch2_t: gconv2 chunks per
    slot tile (len 8). Totals are multiples of WAVE."""
    C1 = sum(nch1_t)
    C2 = sum(nch2_t)
    W1 = C1 // WAVE
    W2 = C2 // WAVE
    nc = bacc.Bacc("TRN2")

    def din(name, shape, dtype=F16):
        return nc.dram_tensor(name, shape, dtype, kind="ExternalInput")

    x_t = din("x_t", [512, R])                  # [x | ko]^T fp16
    pre_t_d = din("pre_t", [512, R])
    ndemb = din("ndemb", [2 * TX, 128])
    name_W = din("name_W", [TX, TX])
    desc_W = din("desc_W", [TX, TX])
    omic_W = din("omic_W", [512, 512])
    fus_nd = din("fus_nd", [2 * TX, 512])
    fus_om = din("fus_om", [512, 512])
    ienc_W = din("ienc_W", [512, 512])
    pre_W = din("pre_W", [512, 512])
    enc_W = din("enc_W", [512, 512])
    gate_W1 = din("gate_W1", [512, 512], F32)
    gw2reg = din("gw2reg", [128, 8], F32)
    bias_pf = din("bias_pf", [128, 26], F32)
    bias_rows = din("bias_rows", [96, 512], F32)
    idx1_d = din("idx1", [128, W1 * WCOLS], I16)
    sel1_d = din("sel1", [C1 * 128, 128], F16)
    idx2_d = din("idx2", [128, W2 * WCOLS], I16)
    sel2_d = din("sel2", [C2 * 128, 128], F16)
    out_d = nc.dram_tensor("out", [1, 2], F32, kind="ExternalOutput")

    agnd_in = nc.dram_tensor("agnd_in", [512, 128], F16)
    agnd_out = nc.dram_tensor("agnd_out", [NCORE * 512, 128], F16, addr_space="Shared")
    ag1_in = nc.dram_tensor("ag1_in", [R, 512], F16)
    ag1_out = nc.dram_tensor("ag1_out", [N, 512], F16, addr_space="Shared")
    m2_loc = nc.dram_tensor("m2_loc", [R, 512], F16)
    m2a_d = nc.dram_tensor("m2a_d", [R, 512], F16)
    rs_in = nc.dram_tensor("rs_in", [8 * 128, 512], F32)
    rs_out = nc.dram_tensor("rs_out", [128, 512], F32)
    RG = [list(range(NCORE))]

    if DEBUG:
        dbg_cross = nc.dram_tensor("dbg_cross", [512, R], F16, kind="ExternalOutput")
        dbg_m1 = nc.dram_tensor("dbg_m1", [R, 512], F16, kind="ExternalOutput")
        dbg_m2 = nc.dram_tensor("dbg_m2", [R, 512], F16, kind="ExternalOutput")
        dbg_zk = nc.dram_tensor("dbg_zk", [128, 512], F32, kind="ExternalOutput")
        dbg_ut = nc.dram_tensor("dbg_ut", [512, R], F16, kind="ExternalOutput")

    with tile.TileContext(nc) as tc:
        with (
            tc.tile_pool(name="pbig", bufs=8) as pbig,
            tc.tile_pool(name="pmed", bufs=1) as pmed,
            tc.tile_pool(name="pw", bufs=1) as pw,
            tc.tile_pool(name="pg", bufs=1) as pg,
            tc.tile_pool(name="psc", bufs=1) as psc,
            tc.tile_pool(name="pp", bufs=1, space="PSUM") as pp,
        ):
            # ---- constants ----
            bpf = psc.tile([128, 26], F32, tag="bpf", bufs=1)
            nc.sync.dma_start(out=bpf[:], in_=bias_pf[:])
            brow_g = psc.tile([1, 512], F32, tag="brow_g", bufs=1)
            nc.sync.dma_start(out=brow_g[:], in_=bias_rows[64:65, :])
            ones = psc.tile([1, 512], F32, tag="ones", bufs=1)
            nc.vector.memset(ones[:], 1.0)
            ident = psc.tile([128, 128], F32, tag="ident", bufs=1)
            make_identity(nc, ident[:])
            idx1 = psc.tile([128, W1 * WCOLS], I16, tag="idx1", bufs=1)
            nc.sync.dma_start(out=idx1[:], in_=idx1_d[:])
            idx2 = psc.tile([128, W2 * WCOLS], I16, tag="idx2", bufs=1)
            nc.sync.dma_start(out=idx2[:], in_=idx2_d[:])

            # ---- big activations (fp16) ----
            xt = []
            for k in range(4):
                t = pbig.tile([128, R], F16, tag="bigA", bufs=8, name=f"xt{k}")
                nc.sync.dma_start(out=t[:], in_=x_t[128 * k:128 * (k + 1), :])
                xt.append(t)

            # ---- ND path (128 entities) ----
            nd_act = []
            for half in range(2):
                W_d = name_W if half == 0 else desc_W
                embs = []
                for ki in range(6):
                    e_ = psc.tile([128, 128], F16, tag="emb", bufs=7,
                                  name=f"emb{half}_{ki}")
                    nc.sync.dma_start(
                        out=e_[:],
                        in_=ndemb[half * TX + 128 * ki: half * TX + 128 * (ki + 1), :])
                    embs.append(e_)
                for mo in range(6):
                    ps = pp.tile([128, 512], F32, tag="ps_mm", bufs=2, space="PSUM")
                    wstrip = pw.tile([128, 6, 128], F16, tag="wnd6", bufs=3)
                    nc.sync.dma_start(
                        out=wstrip[:],
                        in_=W_d[:, 128 * mo:128 * (mo + 1)].rearrange(
                            "(ki p) m -> p ki m", p=128))
                    for ki in range(6):
                        nc.tensor.matmul(ps[:, :128], lhsT=wstrip[:, ki, :],
                                         rhs=embs[ki][:],
                                         start=(ki == 0), stop=(ki == 5))
                    a = psc.tile([128, 128], F16, tag="ndact", bufs=12,
                                 name=f"ndact{half}_{mo}")
                    bt = bpf[:, 6 * half + mo:6 * half + mo + 1]
                    tnd = psc.tile([128, 128], F32, tag="tmpnd", bufs=2)
                    nc.vector.tensor_scalar(out=tnd[:], in0=ps[:, :128], scalar1=bt,
                                            scalar2=SLOPE, op0=ALU.add, op1=ALU.mult)
                    nc.vector.tensor_scalar(out=a[:], in0=ps[:, :128], scalar1=bt,
                                            scalar2=None, op0=ALU.add)
                    nc.vector.tensor_tensor(out=a[:], in0=a[:], in1=tnd[:], op=ALU.max)
                    nd_act.append(a)
            for mo in range(4):
                ps = pp.tile([128, 512], F32, tag="ps_mm", bufs=2, space="PSUM")
                wstrip = pw.tile([128, 12, 128], F16, tag="wnd12", bufs=2)
                nc.sync.dma_start(
                    out=wstrip[:],
                    in_=fus_nd[:, 128 * mo:128 * (mo + 1)].rearrange(
                        "(ki p) m -> p ki m", p=128))
                for ki in range(12):
                    nc.tensor.matmul(ps[:, :128], lhsT=wstrip[:, ki, :],
                                     rhs=nd_act[ki][:],
                                     start=(ki == 0), stop=(ki == 11))
                r_ = psc.tile([128, 128], F16, tag="ndres", bufs=4, name=f"ndres{mo}")
                nc.vector.tensor_copy(out=r_[:], in_=ps[:, :128])
                nc.sync.dma_start(out=agnd_in[128 * mo:128 * (mo + 1), :], in_=r_[:])
            nc.gpsimd.collective_compute(
                "AllGather", ALU.bypass, replica_groups=RG,
                ins=[agnd_in[:]], outs=[agnd_out[:]])

            # ---- omic + fus -> cross_c^T (fp16) ----
            womic = [pw.tile([128, 512], F16, tag="wres", bufs=12, name=f"womic{k}")
                     for k in range(4)]
            wfom = [pw.tile([128, 512], F16, tag="wres", bufs=12, name=f"wfom{k}")
                    for k in range(4)]
            for k in range(4):
                nc.sync.dma_start(out=womic[k][:], in_=omic_W[128 * k:128 * (k + 1), :])
                nc.sync.dma_start(out=wfom[k][:], in_=fus_om[128 * k:128 * (k + 1), :])
            cross = [pbig.tile([128, R], F16, tag="bigA", bufs=8, name=f"cross{k}")
                     for k in range(4)]
            for j in range(4):
                sl = slice(512 * j, 512 * (j + 1))
                om_j = []
                for k in range(4):
                    ps = pp.tile([128, 512], F32, tag="ps_mm", bufs=2, space="PSUM")
                    for ki in range(4):
                        nc.tensor.matmul(ps[:], lhsT=womic[ki][:, 128 * k:128 * (k + 1)],
                                         rhs=xt[ki][:, sl], start=(ki == 0), stop=(ki == 3))
                    a = pmed.tile([128, 512], F16, tag="omj", bufs=4)
                    bt = bpf[:, 12 + k:13 + k]
                    tom = pmed.tile([128, 512], F32, tag="tmpom", bufs=2)
                    nc.vector.tensor_scalar(out=tom[:], in0=ps[:], scalar1=bt,
                                            scalar2=SLOPE, op0=ALU.add, op1=ALU.mult)
                    nc.vector.tensor_scalar(out=a[:], in0=ps[:], scalar1=bt,
                                            scalar2=None, op0=ALU.add)
                    nc.vector.tensor_tensor(out=a[:], in0=a[:], in1=tom[:], op=ALU.max)
                    om_j.append(a)
                for k in range(4):
                    ps = pp.tile([128, 512], F32, tag="ps_mm", bufs=2, space="PSUM")
                    for ki in range(4):
                        nc.tensor.matmul(ps[:], lhsT=wfom[ki][:, 128 * k:128 * (k + 1)],
                                         rhs=om_j[ki][:], start=(ki == 0),
                                         stop=(ki == 3))
                    nc.vector.tensor_copy(out=cross[k][:, sl], in_=ps[:])
                    # + tiled cross_nd via accumulate-DMA (fus_b asserted zero)
                    r0 = 4 * (j % 2)
                    src = agnd_out[:].rearrange(
                        "(r q p) c -> q p r c", r=NCORE, q=4)[k, :, r0:r0 + 4, :]
                    nc.gpsimd.dma_start(
                        out=cross[k][:, sl].rearrange("p (r c) -> p r c", r=4),
                        in_=src, accum_op=ALU.add)
            nc.sync.dma_start(out=cross[3][127:128, :], in_=x_t[511:512, :])
            if DEBUG:
                for k in range(4):
                    nc.sync.dma_start(out=dbg_cross[128 * k:128 * (k + 1), :],
                                      in_=cross[k][:])

            # ---- m1 (row-major fp16) + AG1 ----
            wienc = [pw.tile([128, 512], F16, tag="wres", bufs=12, name=f"wienc{k}")
                     for k in range(4)]
            for k in range(4):
                nc.sync.dma_start(out=wienc[k][:], in_=ienc_W[128 * k:128 * (k + 1), :])
            for t in range(NT):
                tsl = slice(128 * t, 128 * (t + 1))
                ps = pp.tile([128, 512], F32, tag="ps_mm", bufs=2, space="PSUM")
                for ki in range(4):
                    nc.tensor.matmul(ps[:], lhsT=cross[ki][:, tsl], rhs=wienc[ki][:],
                                     start=(ki == 0), stop=(ki == 3))
                h = pmed.tile([128, 512], F16, tag="m1h", bufs=3)
                nc.vector.tensor_copy(out=h[:], in_=ps[:])
                nc.sync.dma_start(out=ag1_in[tsl, :], in_=h[:])
            nc.gpsimd.collective_compute(
                "AllGather", ALU.bypass, replica_groups=RG,
                ins=[ag1_in[:]], outs=[ag1_out[:]])
            if DEBUG:
                nc.sync.dma_start(out=dbg_m1[:], in_=ag1_in[:])

            # ---- zpre (in place on xt; pre_b asserted zero) ----
            wpre = [pw.tile([128, 512], F16, tag="wres", bufs=12, name=f"wpre{k}")
                    for k in range(4)]
            for k in range(4):
                nc.sync.dma_start(out=wpre[k][:], in_=pre_W[128 * k:128 * (k + 1), :])
            for j in range(4):
                sl = slice(512 * j, 512 * (j + 1))
                pre_j = []
                for ki in range(4):
                    s = pmed.tile([128, 512], F16, tag="prestream", bufs=4)
                    nc.sync.dma_start(out=s[:], in_=pre_t_d[128 * ki:128 * (ki + 1), sl])
                    pre_j.append(s)
                for k in range(4):
                    ps = pp.tile([128, 512], F32, tag="ps_mm", bufs=2, space="PSUM")
                    for ki in range(4):
                        nc.tensor.matmul(ps[:], lhsT=wpre[ki][:, 128 * k:128 * (k + 1)],
                                         rhs=pre_j[ki][:], start=(ki == 0),
                                         stop=(ki == 3))
                    nc.vector.tensor_tensor(out=xt[k][:, sl], in0=xt[k][:, sl],
                                            in1=ps[:], op=ALU.add)

            # ---- m2a = zpre @ enc_W (fp16 stash to DRAM) ----
            wenc = [pw.tile([128, 512], F16, tag="wres", bufs=12, name=f"wenc{k}")
                    for k in range(4)]
            for k in range(4):
                nc.sync.dma_start(out=wenc[k][:], in_=enc_W[128 * k:128 * (k + 1), :])
            for t in range(NT):
                tsl = slice(128 * t, 128 * (t + 1))
                ps = pp.tile([128, 512], F32, tag="ps_mm", bufs=2, space="PSUM")
                for ki in range(4):
                    nc.tensor.matmul(ps[:], lhsT=xt[ki][:, tsl], rhs=wenc[ki][:],
                                     start=(ki == 0), stop=(ki == 3))
                h = pmed.tile([128, 512], F16, tag="m2a", bufs=3)
                nc.vector.tensor_copy(out=h[:], in_=ps[:])
                nc.sync.dma_start(out=m2a_d[tsl, :], in_=h[:])

            # ---- generic gather+scatter ----
            def scatter(src_dram, idx_t, sel_d, sel_dt, nchunks, tile_bounds,
                        psum_tag, gbufs_n):
                gbufs = {}
                sbufs = {}
                out_psums = []
                ps = None
                sel_r = sel_d[:].rearrange("(c e) d -> e c d", e=128)
                for i in range(nchunks):
                    w, slot = divmod(i, WAVE)
                    if slot == 0:
                        nch_w = min(WAVE, nchunks - w * WAVE)
                        assert nch_w == WAVE, "waves must divide chunk count"
                        g = pg.tile([128, WAVE, 512], sel_dt, tag="gath",
                                    bufs=gbufs_n)
                        nc.gpsimd.dma_gather(
                            g[:, :nch_w, :], src_dram[:],
                            idx_t[:, WCOLS * w:WCOLS * w + nch_w * 8],
                            nch_w * 128, nch_w * 128, 512,
                            single_packet=False)
                        gbufs[w] = g
                        sw = pg.tile([128, WAVE, 128], sel_dt, tag="selw",
                                     bufs=gbufs_n)
                        nc.sync.dma_start(
                            out=sw[:, :nch_w, :],
                            in_=sel_r[:, WAVE * w:WAVE * w + nch_w, :])
                        sbufs[w] = sw
                    t_id, first, last = tile_bounds[i]
                    if first:
                        ps = pp.tile([128, 512], F32, tag=psum_tag, bufs=2,
                                     space="PSUM")
                    nc.tensor.matmul(ps[:], lhsT=sbufs[w][:, slot, :],
                                     rhs=gbufs[w][:, slot, :],
                                     start=first, stop=last)
                    if last:
                        out_psums.append((t_id, ps))
                return out_psums

            bounds1 = []
            for t_id, nch in enumerate(nch1_t):
                for j in range(nch):
                    bounds1.append((t_id, j == 0, j == nch - 1))
            seg1 = scatter(ag1_out, idx1, sel1_d, F16, C1, bounds1, "ps_seg", 5)

            # ---- u^T (fp16) via PE transpose + lrelu(+ienc_b) ----
            ut = [pbig.tile([128, R], F16, tag="bigA", bufs=8, name=f"ut{k}")
                  for k in range(4)]
            for q in range(4):
                pst = [pp.tile([128, 512], F32, tag="ps_ut", bufs=4, space="PSUM",
                               name=f"pst{q}_{k_}") for k_ in range(4)]
                for tt in range(4):
                    t_id, ps = seg1[4 * q + tt]
                    m1t = pmed.tile([128, 512], F16, tag="m1h", bufs=3)
                    nc.sync.dma_start(out=m1t[:],
                                      in_=ag1_in[128 * t_id:128 * (t_id + 1), :])
                    useg = pmed.tile([128, 512], F32, tag="useg", bufs=3)
                    nc.vector.tensor_tensor(out=useg[:], in0=ps[:],
                                            in1=m1t[:], op=ALU.add)
                    for k in range(4):
                        nc.tensor.transpose(
                            out=pst[k][:, 128 * tt:128 * (tt + 1)],
                            in_=useg[:, 128 * k:128 * (k + 1)], identity=ident[:])
                for k in range(4):
                    sl = slice(512 * q, 512 * (q + 1))
                    tmp = pmed.tile([128, 512], F32, tag="lrtmp", bufs=2)
                    nc.vector.tensor_scalar(
                        out=tmp[:], in0=pst[k][:], scalar1=bpf[:, 16 + k:17 + k],
                        scalar2=SLOPE, op0=ALU.add, op1=ALU.mult)
                    nc.vector.tensor_scalar(
                        out=ut[k][:, sl], in0=pst[k][:], scalar1=bpf[:, 16 + k:17 + k],
                        scalar2=None, op0=ALU.add)
                    nc.vector.tensor_tensor(
                        out=ut[k][:, sl], in0=ut[k][:, sl], in1=tmp[:], op=ALU.max)
            if DEBUG:
                for k in range(4):
                    nc.sync.dma_start(out=dbg_ut[128 * k:128 * (k + 1), :], in_=ut[k][:])

            # ---- m2 = m2a + u @ enc_W (fp32, stays local) ----
            for t in range(NT):
                tsl = slice(128 * t, 128 * (t + 1))
                ps = pp.tile([128, 512], F32, tag="ps_mm", bufs=2, space="PSUM")
                for ki in range(4):
                    nc.tensor.matmul(ps[:], lhsT=ut[ki][:, tsl], rhs=wenc[ki][:],
                                     start=(ki == 0), stop=(ki == 3))
                m2a_t = pmed.tile([128, 512], F16, tag="m2a", bufs=3)
                nc.sync.dma_start(out=m2a_t[:], in_=m2a_d[tsl, :])
                h = pmed.tile([128, 512], F16, tag="m2h", bufs=3)
                nc.vector.tensor_tensor(out=h[:], in0=ps[:], in1=m2a_t[:], op=ALU.add)
                nc.sync.dma_start(out=m2_loc[tsl, :], in_=h[:])
            if DEBUG:
                nc.sync.dma_start(out=dbg_m2[:], in_=m2_loc[:])

            # ---- gconv2: source-side partials over 1024 slots + ReduceScatter ----
            bounds2 = []
            for t_id, nch in enumerate(nch2_t):
                for j in range(nch):
                    bounds2.append((t_id, j == 0, j == nch - 1))
            seg2 = scatter(m2_loc, idx2, sel2_d, F16, C2, bounds2, "ps_seg", 5)
            for t_id, ps in seg2:
                pc = pmed.tile([128, 512], F32, tag="useg", bufs=3)
                nc.vector.tensor_copy(out=pc[:], in_=ps[:])
                nc.sync.dma_start(out=rs_in[128 * t_id:128 * (t_id + 1), :], in_=pc[:])
            nc.gpsimd.collective_compute(
                "ReduceScatter", ALU.add, replica_groups=RG,
                ins=[rs_in[:]], outs=[rs_out[:]])

            # ---- zk^T + readout ----
            zkr = pmed.tile([128, 512], F32, tag="useg", bufs=3)
            nc.sync.dma_start(out=zkr[:], in_=rs_out[:])
            if DEBUG:
                nc.sync.dma_start(out=dbg_zk[:], in_=zkr[:])
            ps_zt = pp.tile([128, 512], F32, tag="ps_ut", bufs=4, space="PSUM")
            for k in range(4):
                nc.tensor.transpose(out=ps_zt[:, 128 * k:128 * (k + 1)],
                                    in_=zkr[:, 128 * k:128 * (k + 1)], identity=ident[:])
            zkt = pmed.tile([128, 512], F32, tag="zkt", bufs=1)
            for k in range(4):
                sl = slice(128 * k, 128 * (k + 1))
                tmp = pmed.tile([128, 128], F32, tag="lrtmp2", bufs=2)
                nc.vector.tensor_scalar(
                    out=tmp[:], in0=ps_zt[:, sl], scalar1=bpf[:, 20 + k:21 + k],
                    scalar2=SLOPE, op0=ALU.add, op1=ALU.mult)
                nc.vector.tensor_scalar(
                    out=zkt[:, sl], in0=ps_zt[:, sl], scalar1=bpf[:, 20 + k:21 + k],
                    scalar2=None, op0=ALU.add)
                nc.vector.tensor_tensor(
                    out=zkt[:, sl], in0=zkt[:, sl], in1=tmp[:], op=ALU.max)

            wg1 = [pw.tile([128, 512], F32, tag="wres32", bufs=4, name=f"wg1{k}")
                   for k in range(4)]
            for k in range(4):
                nc.sync.dma_start(out=wg1[k][:], in_=gate_W1[128 * k:128 * (k + 1), :])
            w2r = psc.tile([128, 8], F32, tag="w2r", bufs=1)
            nc.sync.dma_start(out=w2r[:], in_=gw2reg[:])
            s1t = pmed.tile([128, 512], F32, tag="s1t", bufs=1)
            for ko_ in range(4):
                ps = pp.tile([128, 512], F32, tag="ps_mm", bufs=2, space="PSUM")
                for ki in range(4):
                    nc.tensor.matmul(ps[:, :128],
                                     lhsT=wg1[ki][:, 128 * ko_:128 * (ko_ + 1)],
                                     rhs=zkt[:, 128 * ki:128 * (ki + 1)],
                                     start=(ki == 0), stop=False)
                nc.tensor.matmul(ps[:, :128],
                                 lhsT=brow_g[:, 128 * ko_:128 * (ko_ + 1)],
                                 rhs=ones[:, :128], start=False, stop=True)
                nc.scalar.activation(s1t[:, 128 * ko_:128 * (ko_ + 1)], ps[:, :128],
                                     ACTF.Tanh)
            ps_sc = pp.tile([128, 512], F32, tag="ps_mm", bufs=2, space="PSUM")
            for ki in range(4):
                nc.tensor.matmul(ps_sc[:1, :128], lhsT=w2r[:, 2 * ki:2 * ki + 1],
                                 rhs=s1t[:, 128 * ki:128 * (ki + 1)],
                                 start=(ki == 0), stop=(ki == 3))
            ps_tr = pp.tile([128, 512], F32, tag="ps_seg", bufs=2, space="PSUM")
            for ki in range(4):
                nc.tensor.matmul(ps_tr[:1, :128], lhsT=w2r[:, 2 * ki + 1:2 * ki + 2],
                                 rhs=zkt[:, 128 * ki:128 * (ki + 1)],
                                 start=(ki == 0), stop=(ki == 3))
            erow = psc.tile([1, 128], F32, tag="erow", bufs=1)
            nc.scalar.activation(erow[:], ps_sc[:1, :128], ACTF.Exp,
                                 bias=bpf[:1, 24:25])
            etrow = psc.tile([1, 128], F32, tag="etrow", bufs=1)
            nc.vector.tensor_tensor(out=etrow[:], in0=erow[:], in1=ps_tr[:1, :128],
                                    op=ALU.mult)
            sums = psc.tile([1, 4], F32, tag="sums", bufs=1)
            nc.vector.tensor_reduce(out=sums[:, 0:2],
                                    in_=etrow[:].rearrange("p (g x) -> p g x", g=2),
                                    axis=AX, op=ALU.add)
            nc.vector.tensor_reduce(out=sums[:, 2:4],
                                    in_=erow[:].rearrange("p (g x) -> p g x", g=2),
                                    axis=AX, op=ALU.add)
            res = psc.tile([1, 4], F32, tag="res", bufs=1)
            nc.vector.reciprocal(out=res[:, 2:4], in_=sums[:, 2:4])
            nc.vector.tensor_tensor(out=res[:, 0:2], in0=sums[:, 0:2],
                                    in1=res[:, 2:4], op=ALU.mult)
            nc.vector.tensor_scalar(out=res[:, 0:2], in0=res[:, 0:2],
                                    scalar1=bpf[:1, 25:26], scalar2=None, op0=ALU.add)
            nc.sync.dma_start(out=out_d[:], in_=res[:, 0:2])

    nc.compile()
    return nc


def _ensure_ntff_hook():
    """Inject antenv.axon_hooks (absent in this image) so trace=True works."""
    import sys, types
    try:
        from antenv.axon_hooks import get_axon_ntff_profile_hook  # noqa
        return
    except ImportError:
        pass
    import antenv
    mod = types.ModuleType("antenv.axon_hooks")
    _state = {"hook": None}
    mod.set_axon_ntff_profile_hook = lambda h: _state.__setitem__("hook", h)
    mod.get_axon_ntff_profile_hook = lambda: _state["hook"]
    sys.modules["antenv.axon_hooks"] = mod
    antenv.axon_hooks = mod
    from trn_agent_boot.trn_boot import _ntff_profile_via_ctypes
    mod.set_axon_ntff_profile_hook(
        _ntff_profile_via_ctypes("/opt/axon/libaxon_pjrt.so"))


# ---------------------------------------------------------------------------
# host wrapper
# ---------------------------------------------------------------------------

def kernel(**inputs):
    f32 = lambda k: np.asarray(inputs[k], np.float32)
    x = f32("x"); pre_x = f32("pre_x")
    edge_index = np.asarray(inputs["edge_index"], np.int64)
    internal_edge_index = np.asarray(inputs["internal_edge_index"], np.int64)
    name_emb = f32("name_embeddings"); desc_emb = f32("desc_embeddings")
    ko_mask = np.asarray(inputs["ko_mask"], np.int64)
    bkm = np.asarray(inputs["batch_ko_masks"], np.int64)
    name_W = f32("name_W"); name_b = f32("name_b")
    desc_W = f32("desc_W"); desc_b = f32("desc_b")
    omic_W = f32("omic_W"); omic_b = f32("omic_b")
    fus_W = f32("fus_W"); fus_b = f32("fus_b")
    pre_W = f32("pre_W"); pre_b = f32("pre_b")
    ienc_W = f32("ienc_W"); ienc_b = f32("ienc_b")
    enc_W = f32("enc_W"); enc_b = f32("enc_b")
    gate_W1 = f32("gate_W1"); gate_b1 = f32("gate_b1")
    gate_W2 = f32("gate_W2"); gate_b2 = f32("gate_b2")
    reg_W = f32("reg_W"); reg_b = f32("reg_b")

    assert not fus_b.any() and not pre_b.any(), \
        "nonzero fus_b/pre_b not supported by this build"

    ko_feat = np.zeros(N, np.float32)
    ko_feat[ko_mask] = 1.0

    # ---- gconv1 edges (dst-sharded; self term added from local m1h) ----
    s1_all, d1_all = internal_edge_index[0], internal_edge_index[1]
    per_core_1 = []
    nch1_t = np.ones(NT, np.int64)
    for c in range(NCORE):
        lo, hi = R * c, R * (c + 1)
        m = (d1_all >= lo) & (d1_all < hi)
        per_core_1.append((s1_all[m], d1_all[m] - lo))
        cnt = np.bincount((d1_all[m] - lo) >> 7, minlength=NT)
        nch1_t = np.maximum(nch1_t, -(-cnt // 128))
    pad1 = (-int(nch1_t.sum())) % WAVE
    nch1_t[NT - 1] += pad1
    nch1_t = tuple(int(v) for v in nch1_t)

    # ---- gconv2: source-sharded edges into the 1024 global KO slots ----
    slot_row = (bkm + np.arange(B)[:, None] * NE).reshape(-1)   # [1024]
    row2slots = {}
    for s_, r_ in enumerate(slot_row):
        row2slots.setdefault(int(r_), []).append(s_)
    s2_all, d2_all = edge_index[0], edge_index[1]
    m2mask = np.isin(d2_all, slot_row)
    per_core_2 = []
    nch2_t = np.ones(8, np.int64)
    for c in range(NCORE):
        lo, hi = R * c, R * (c + 1)
        ss, ds = [], []
        for r_, sl_ in row2slots.items():
            if lo <= r_ < hi:
                for s_ in sl_:
                    ss.append(r_ - lo); ds.append(s_)
        mm = m2mask & (s2_all >= lo) & (s2_all < hi)
        for u, v in zip(s2_all[mm], d2_all[mm]):
            for s_ in row2slots[int(v)]:
                ss.append(int(u) - lo); ds.append(s_)
        src = np.array(ss, np.int64); dstl = np.array(ds, np.int64)
        per_core_2.append((src, dstl))
        cnt = np.bincount(dstl >> 7, minlength=8)
        nch2_t = np.maximum(nch2_t, -(-cnt // 128))
    pad2 = (-int(nch2_t.sum())) % WAVE
    nch2_t[7] += pad2
    nch2_t = tuple(int(v) for v in nch2_t)

    nc = _build(nch1_t, nch2_t)

    f16 = np.float16
    omic_Wp = _pad_w(omic_W, 512, 512)
    fus_ndp = _pad_w(fus_W[:2 * TX], 2 * TX, 512)
    fus_omp = _pad_w(fus_W[2 * TX:], 512, 512)
    bias_pf = np.zeros((128, 26), np.float32)
    bias_pf[:, 0:6] = name_b.reshape(6, 128).T
    bias_pf[:, 6:12] = desc_b.reshape(6, 128).T
    bias_pf[:, 12:16] = _pad_w(omic_b[:, None], 512, 1).reshape(4, 128).T
    bias_pf[:, 16:20] = ienc_b.reshape(4, 128).T
    bias_pf[:, 20:24] = enc_b.reshape(4, 128).T
    bias_pf[:, 24] = float(gate_b2.reshape(-1)[0])
    bias_pf[:, 25] = float(reg_b.reshape(-1)[0])
    bias_rows = np.zeros((96, 512), np.float32)
    bias_rows[64, :] = gate_b1
    gw2 = np.concatenate([gate_W2, reg_W], axis=1).astype(np.float32)
    gw2 = np.ascontiguousarray(
        gw2.reshape(4, 128, 2).transpose(1, 0, 2).reshape(128, 8))

    shared = dict(
        name_W=name_W.astype(f16), desc_W=desc_W.astype(f16),
        omic_W=omic_Wp.astype(f16), fus_nd=fus_ndp.astype(f16),
        fus_om=fus_omp.astype(f16), ienc_W=ienc_W.astype(f16),
        pre_W=pre_W.astype(f16), enc_W=enc_W.astype(f16),
        gate_W1=gate_W1, gw2reg=gw2, bias_pf=bias_pf, bias_rows=bias_rows,
    )

    in_maps = []
    for c in range(NCORE):
        lo, hi = R * c, R * (c + 1)
        x_t = np.concatenate([x[lo:hi].T, ko_feat[None, lo:hi]], 0)
        pre_t = np.concatenate([pre_x[lo:hi].T, ko_feat[None, lo:hi]], 0)
        ndemb = np.concatenate(
            [name_emb[128 * c:128 * (c + 1)].T, desc_emb[128 * c:128 * (c + 1)].T], 0)
        i1, dv1 = _chunk_edges_per_tile(*per_core_1[c], nch1_t)
        i2, dv2 = _chunk_edges_per_tile(*per_core_2[c], nch2_t)
        in_maps.append(dict(
            x_t=np.ascontiguousarray(x_t).astype(f16),
            pre_t=np.ascontiguousarray(pre_t).astype(f16),
            ndemb=np.ascontiguousarray(ndemb).astype(f16),
            idx1=_wrap_idx_waves(i1),
            sel1=_sel_from_dstv(dv1, f16),
            idx2=_wrap_idx_waves(i2),
            sel2=_sel_from_dstv(dv2, f16),
            **shared,
        ))

    if TRACE:
        _ensure_ntff_hook()
    res = run_bass_kernel_spmd(nc, in_maps, core_ids=list(range(NCORE)),
                               trace=TRACE, **(TRACE_KW or {}))
    kernel._last = res
    out = np.zeros(B, np.float32)
    for c in range(NCORE):
        out[2 * c:2 * c + 2] = res.results[c]["out"][0]
    return out
